# revision 55
# baseline (speedup 1.0000x reference)
"""Trainium2 Bass kernel for ConvPixelToCapsules (conv -> 3-iter dynamic routing).

Strategy (hardcoded for x[8,32,8,32,32], conv_w[256,8,3,3], bias[32,8,1,1]):
  - Host precomputes im2col patches per batch element with two extra tricks:
    a 33rd "channel" slot holding sum_ci(x) (conv linearity gives iteration
    1's uniform-route preactivation for free) and a 73rd contraction row
    (1.0 in the xsum slot only) whose weight row is CI*bias, so iteration
    1's preactivation S1 = Uxs/CI comes out of the conv bias-included as a
    single scaled PSUM evacuation.
  - 8 NeuronCores, data-parallel over batch: core k owns batch element k.
  - Per core: 8 tiles of 128 output pixels, software-pipelined in pairs
    three groups deep: tail(g-1) || main(g) || conv(g+1).  Phases:
      conv(t): patches DMA, conv matmuls, Act PSUM evacuation, squash1
      main(t): d1, softmax2, s2, squash2, d2, exp3 (iterations 1+2)
      tail(t): softmax3 (from E3 in SBUF), s3, squash3, transpose, out DMA
  - Emission order = per-engine program order (queues are in-order), so
    drain() is a greedy list scheduler: each generator segment carries an
    {engine: est_ns} cost tag and the scheduler advances the generator
    whose segment finishes earliest on a virtual per-engine timeline.
  - Votes live in SBUF as [pixel-partition; (ci,no,co)] bf16; all routing
    products are DVE bf16 2x ops with the last-consumed ci-slices offloaded
    to GPSIMD (so the slow engine never gates the PE accumulation).
  - Both routing contractions run on the PE as accumulating identity
    matmuls; the d-contraction runs per ci-half into a 1-bank PSUM tile
    whose exp is taken immediately, freeing the bank (D never persists:
    iteration 3 uses exp(D1+D2) = E2*exp(D2), computed at main-end).
  - Softmax sums over co run on the PE into a shared qacc bank; squash
    computes sqrt via Quake-rsqrt + Newton on DVE bit ops so the Act engine
    only ever needs Copy+Exp (one act-table load for the whole program).
  - PSUM budget (8 banks): conv pv ring 2, D-half ring 3, S ring 2, qacc 1.
"""

import numpy as np

BS, CI, NI, H, W = 8, 32, 8, 32, 32
CO, NO = 32, 8
NPIX = H * W            # 1024
TILES = 8               # tiles of 128 pixels per batch element
TP = 128                # pixels per tile (on partitions)
K = 73                  # ni*3*3 contraction + bias row
SLOTS = CI + 1          # 32 ci + xsum slot
OUTCH = NO * CO         # 256, (no, co) order
QK = 0x5F3759DF         # Quake rsqrt seed constant
HCI = CI // 2           # 16, ci-half for D tiles

CFG = {
    "warmup": 40,
    "newton12": 1,          # Newton iterations for squash 1-2
    "newton3": 1,           # Newton iterations for final squash
    "pool_ci_d": 2,         # ci (of 16, in half 1 only) of d1-products on GPSIMD
    "pool_ci_d2": 8,        # ci (of 16, half 1) of d2-products on GPSIMD
    "pool_ci_s": 8,         # trailing ci (of 32) of s-products on GPSIMD
    "pool_ci_r": 15,        # trailing ci (of 32) of softmax R-product on GPSIMD
    "evac_pool": 0,         # (GPSIMD cannot access PSUM: must stay 0)
    "evac_dve": 0,          # ... how many on DVE
    "head_evac_dve": 0,     # evacs on DVE for tiles 0-1 (pipeline fill)
    "acc_chunk": 4,         # s-phase accum matmuls per segment
    "red_pool": 0,          # (unsupported: GPSIMD reduce is partition-axis only)
    "emul_pool": 0,         # iteration-3 E2 multiply on GPSIMD
    "s1bf": 1,              # S1 in bf16 (V1-multiply gets DVE 2x)
    "sq_pool": 0,           # squash Quake-rsqrt chain on GPSIMD
    "gated_tail": 0,        # last group's tails share their mains' drain
    "sq1_conv": 1,          # emit squash1 inside conv_tile (early Act slot)
    "d2_dma": 0,            # iteration-2 d-contraction as SWDGE accum-DMA tree
    "dsum": 0,              # hold D1 in PSUM; d2 accumulates onto it
    "handoff": 125,         # scheduler estimate of cross-engine sem latency
    "seed": 0,              # scheduler jitter seed (0 = no jitter)
    "jit": 0.15,            # jitter amplitude on segment estimates
    "votes_bufs": 5,
    "big_bufs": 3,
    "pat_bufs": 3,
    "pconv_bufs": 2,
    "pd_bufs": 3,
    "ps_bufs": 2,
}

_BUILT = {}

# segment cost helpers (ns estimates for the emission scheduler)
def _dve(elems, f32=False):
    return {"dve": elems / (0.96 if f32 else 1.92) + 130}

def _pool(elems):
    return {"pool": elems / 0.504 + 150}

def _act(elems):
    return {"act": elems / 1.2 + 220}

def _pe(cols, n=1):
    return {"pe": cols * 0.417 + n * 6}

def _merge(*tags):
    out = {}
    for t in tags:
        for k, v in t.items():
            out[k] = out.get(k, 0.0) + v
    return out


def _host_prep(x, conv_w, bias):
    x = np.asarray(x, np.float32)
    conv_w = np.asarray(conv_w, np.float32)
    bias = np.asarray(bias, np.float32)
    x_pad = np.pad(x, ((0, 0), (0, 0), (0, 0), (1, 1), (1, 1)))
    x_aug = np.concatenate([x_pad, x_pad.sum(1, keepdims=True)], axis=1)
    wv = np.lib.stride_tricks.sliding_window_view(x_aug, (3, 3), axis=(3, 4))
    import ml_dtypes
    cdt_np = ml_dtypes.bfloat16
    patches = np.ascontiguousarray(
        wv.transpose(0, 2, 5, 6, 1, 3, 4).reshape(BS, K - 1, SLOTS, NPIX)
    ).astype(cdt_np)
    # row 72: 1.0 in the xsum slot only -> the conv adds CI*bias to Uxs,
    # making iteration 1's preactivation a pure scaled copy at evac time.
    brow = np.zeros((BS, 1, SLOTS, NPIX), dtype=cdt_np)
    brow[:, :, CI, :] = 1.0
    patches = np.concatenate([patches, brow], axis=1)
    w_m = np.ascontiguousarray(
        conv_w.reshape(CO, NO, NI, 3, 3).transpose(2, 3, 4, 1, 0)
        .reshape(K - 1, OUTCH)
    ).astype(cdt_np)
    w_m = np.concatenate(
        [w_m,
         (CI * bias[:, :, 0, 0].T.reshape(1, OUTCH)).astype(cdt_np)], axis=0)
    bias_bc = np.broadcast_to(
        bias[:, :, 0, 0].T.reshape(1, OUTCH), (128, OUTCH)
    ).astype(np.float32)
    ident = np.eye(128, dtype=np.float32)
    identb = np.eye(128, dtype=cdt_np)
    return patches, w_m, bias_bc, ident, identb


def _build_nc():
    def _freeze(v):
        if isinstance(v, (list, tuple)):
            return tuple(_freeze(x) for x in v)
        return v
    key = ("nc",) + tuple(sorted((k, _freeze(v)) for k, v in CFG.items()))
    if key in _BUILT:
        return _BUILT[key]
    import concourse.bacc as bacc
    import concourse.tile as tile
    import concourse.mybir as mybir

    f32 = mybir.dt.float32
    bf16 = mybir.dt.bfloat16
    u32 = mybir.dt.uint32
    AF = mybir.ActivationFunctionType
    OP = mybir.AluOpType
    AX = mybir.AxisListType

    nc = bacc.Bacc("TRN2", target_bir_lowering=False, debug=False, num_devices=8)

    patches_d = nc.dram_tensor("patches", [K, SLOTS, NPIX], bf16, kind="ExternalInput")
    w_d = nc.dram_tensor("w", [K, OUTCH], bf16, kind="ExternalInput")
    bias_d = nc.dram_tensor("bias", [128, OUTCH], f32, kind="ExternalInput")
    ident_d = nc.dram_tensor("ident", [128, 128], f32, kind="ExternalInput")
    identb_d = nc.dram_tensor("identb", [128, 128], bf16, kind="ExternalInput")
    out_d = nc.dram_tensor("out", [2, 128, NPIX], f32, kind="ExternalOutput")

    with tile.TileContext(nc) as tc:
        with (
            tc.tile_pool(name="const", bufs=1) as const,
            tc.tile_pool(name="pat", bufs=CFG["pat_bufs"]) as patp,
            tc.tile_pool(name="votes", bufs=CFG["votes_bufs"]) as votesp,
            tc.tile_pool(name="s1", bufs=4) as s1p,
            tc.tile_pool(name="big", bufs=CFG["big_bufs"]) as bigp,
            tc.tile_pool(name="state", bufs=4) as statep,
            tc.tile_pool(name="ep", bufs=3) as ep,
            tc.tile_pool(name="obuf", bufs=1) as obufp,
            tc.tile_pool(name="pconv", bufs=CFG["pconv_bufs"], space="PSUM") as pconv,
            tc.tile_pool(name="pd", bufs=CFG["pd_bufs"], space="PSUM") as pdp,
            tc.tile_pool(name="ps", bufs=CFG["ps_bufs"], space="PSUM") as psp,
            tc.tile_pool(name="pq", bufs=1, space="PSUM") as pqp,
        ):
            w_sb = const.tile([K, OUTCH], bf16)
            nc.sync.dma_start(w_sb[:], w_d.ap())
            bias_sb = const.tile([128, OUTCH], f32)
            nc.sync.dma_start(bias_sb[:], bias_d.ap())
            ident_sb = const.tile([128, 128], f32)
            nc.sync.dma_start(ident_sb[:], ident_d.ap())
            identb_sb = const.tile([128, 128], bf16)
            nc.sync.dma_start(identb_sb[:], identb_d.ap())
            ones1 = const.tile([1, 128], f32)
            nc.gpsimd.memset(ones1[:], 1.0)
            qc = const.tile([128, 2 * CO], u32)
            nc.gpsimd.memset(qc[:], QK)

            ob = [
                obufp.tile([128, NPIX], f32, tag=f"ob{h}", name=f"ob{h}")
                for h in range(2)
            ]

            # PE p-state warmup: the tensor engine needs ~3us of continuous
            # work to reach full clock; a burst of dependency-free matmuls
            # fills the initial patches-DMA window.
            warm = pqp.tile([128, 128], f32, tag="qa", name="warm")
            for _ in range(CFG["warmup"]):
                nc.tensor.matmul(
                    warm[:, :64], identb_sb[:], identb_sb[:, :64],
                    start=True, stop=True, skip_group_check=True,
                )

            def conv_tile(t):
                # votes for 128 pixels; Uxs slot first so iteration 1 can
                # start early; ci-pairs share one PSUM bank so evacuation
                # runs as double-width copies.  squash1 runs here (it only
                # needs S1, ready after the first matmul) so its Act square
                # isn't queued behind all 16 in-order evacuations.
                pt = patp.tile([K, SLOTS, TP], bf16, tag="pt", name=f"pt{t}")
                nc.sync.dma_start(
                    pt[:, CI, :], patches_d.ap()[:, CI, t * TP : (t + 1) * TP]
                )
                for dq in range(4):
                    qs = slice(dq * 8, (dq + 1) * 8)
                    nc.sync.dma_start(
                        pt[:, qs, :],
                        patches_d.ap()[:, qs, t * TP : (t + 1) * TP],
                    )
                U = votesp.tile([128, CI, NO, CO], bf16, tag="U", name=f"U{t}")
                S1 = s1p.tile([128, OUTCH], bf16 if CFG["s1bf"] else f32,
                              tag="S1", name=f"S1{t}")
                conv_tile.out[t] = (U, S1)
                pvx = pconv.tile([128, 2 * OUTCH], f32, tag="pv", name=f"pvx{t}")
                nc.tensor.matmul(
                    pvx[:, :OUTCH], pt[:, CI, :], w_sb[:], start=True, stop=True
                )
                nc.scalar.activation(S1[:], pvx[:, :OUTCH], AF.Copy,
                                     0.0, scale=1.0 / CI)
                yield _merge(_pe(256), _act(256))
                if CFG["sq1_conv"]:
                    S1v = S1[:].rearrange("p (n c) -> p n c", n=NO)
                    yield from squash(t, S1v, 1, bf16, CFG["newton12"],
                                      Sb=S1v if CFG["s1bf"] else None)
                    conv_tile.v1[t] = squash.out
                nd, npo = CFG["evac_dve"], CFG["evac_pool"]
                if t < 2:
                    # pipeline fill: DVE is idle during the first convs, so
                    # splitting the evacuation halves the serial evac wall
                    nd = max(nd, CFG["head_evac_dve"])
                for c in range(CI // 2):
                    pv = pconv.tile([128, 2 * OUTCH], f32, tag="pv",
                                    name=f"pv{t}_{c}")
                    nc.tensor.matmul(
                        pv[:, :OUTCH], pt[:, 2 * c, :], w_sb[:],
                        start=True, stop=True,
                    )
                    nc.tensor.matmul(
                        pv[:, OUTCH:], pt[:, 2 * c + 1, :], w_sb[:],
                        start=True, stop=True,
                    )
                    dst = U[:, 2 * c : 2 * c + 2].rearrange(
                        "p c n o -> p (c n o)"
                    )
                    if c < nd:
                        nc.vector.tensor_copy(dst, pv[:])
                        tag = _dve(512, f32=True)
                    elif c < nd + npo:
                        nc.gpsimd.tensor_copy(dst, pv[:])
                        tag = _pool(512)
                    else:
                        nc.scalar.copy(dst, pv[:])
                        tag = _act(512)
                    yield _merge(_pe(512, 2), tag)
            conv_tile.out = {}
            conv_tile.v1 = {}

            def emit_out(t, V, SBt):
                # transposes land in the upper half of the final iteration's
                # S PSUM bank (no separate PSUM pool needed)
                Vf = V[:].rearrange("p n c -> p (n c)")
                for h in range(2):
                    tp = SBt[:, OUTCH + h * 128 : OUTCH + (h + 1) * 128]
                    nc.tensor.transpose(
                        tp, Vf[:, h * 128 : (h + 1) * 128], ident_sb[:]
                    )
                    nc.scalar.copy(ob[h][:, t * TP : (t + 1) * TP], tp)
                    nc.sync.dma_start(
                        out_d.ap()[h][:, t * TP : (t + 1) * TP],
                        ob[h][:, t * TP : (t + 1) * TP],
                    )

            def squash(t, S, it, out_dtype, newton, Sb=None):
                # S: [128, NO, CO] f32 (SBUF or PSUM view) -> V [128, NO, CO]
                # scl = sqrt(n)/(1+n) via Quake rsqrt (no act tables needed).
                # Sb: optional bf16 copy of S (keeps the V-multiply in DVE 2x
                # mode); the norm reduce runs on GPSIMD to spare DVE.
                sq = statep.tile([128, NO, CO], f32, tag="sq", name=f"sq{t}_{it}")
                nc.scalar.square(sq[:], S)
                nsq = statep.tile([128, CO], f32, tag="nsq", name=f"nsq{t}_{it}")
                if CFG["red_pool"]:
                    nc.gpsimd.tensor_reduce(
                        nsq[:], sq[:].transpose([0, 2, 1]), axis=AX.X, op=OP.add
                    )
                    yield _merge(_act(256), _pool(256))
                else:
                    nc.vector.tensor_reduce(
                        nsq[:], sq[:].transpose([0, 2, 1]), axis=AX.X, op=OP.add
                    )
                    yield _merge(_act(256), _dve(256, f32=True))
                eng = nc.gpsimd if CFG["sq_pool"] else nc.vector
                sh = statep.tile([128, CO], u32, tag="sh", name=f"sh{t}_{it}")
                eng.tensor_scalar(
                    sh[:], nsq[:].bitcast(u32), 1, None,
                    op0=OP.logical_shift_right,
                )
                y = statep.tile([128, CO], f32, tag="y", name=f"y{t}_{it}")
                eng.tensor_tensor(
                    y[:].bitcast(u32), qc[:, :CO], sh[:], op=OP.subtract
                )
                den = statep.tile([128, CO], f32, tag="den", name=f"den{t}_{it}")
                eng.tensor_scalar_add(den[:], nsq[:], 1.0)
                rcd = statep.tile([128, CO], f32, tag="rcd", name=f"rcd{t}_{it}")
                nc.vector.reciprocal(rcd[:], den[:])
                tq = statep.tile([128, CO], f32, tag="tq", name=f"tq{t}_{it}")
                for _ in range(newton):
                    eng.tensor_mul(tq[:], y[:], y[:])
                    eng.tensor_mul(tq[:], tq[:], nsq[:])
                    eng.tensor_scalar(
                        tq[:], tq[:], -0.5, 1.5, op0=OP.mult, op1=OP.add
                    )
                    eng.tensor_mul(y[:], y[:], tq[:])
                yield {"pool" if CFG["sq_pool"] else "dve": 900}
                # scl = nsq * y * rcd  (= sqrt(nsq)/(1+nsq))
                sdt = bf16 if Sb is not None else f32
                scl = statep.tile([128, CO], sdt, tag=f"scl{sdt}",
                                  name=f"scl{t}_{it}")
                scm = statep.tile([128, CO], f32, tag="scm", name=f"scm{t}_{it}")
                nc.vector.tensor_mul(scm[:], nsq[:], y[:])
                nc.vector.tensor_mul(scl[:], scm[:], rcd[:])
                V = statep.tile([128, NO, CO], out_dtype, tag=f"V{it}",
                                name=f"V{t}_{it}")
                nc.vector.tensor_mul(
                    V[:], S if Sb is None else Sb,
                    scl[:].unsqueeze(1).broadcast_to([128, NO, CO])
                )
                yield _dve(600, f32=(Sb is None))
                squash.out = V

            def s_phase(t, U, R, it):
                # Fused: tmp = U*R (bf16 2x, trailing ci-slice on GPSIMD)
                # pipelined into the PE ci-contraction. Identity stays the
                # stationary, so each matmul is a PSUM-accumulating copy;
                # bias opens the group as a rank-1 ones x bias_row matmul.
                SBt = psp.tile([128, 512], f32, tag="S", name=f"SB{t}_{it}")
                SB = SBt[:, :OUTCH]
                tmp = bigp.tile([128, CI, NO, CO], bf16, tag="tmp",
                                name=f"tmps{it}_{t}")
                facb = R[:].unsqueeze(2).broadcast_to([128, CI, NO, CO])
                nc.tensor.matmul(
                    SB, ones1[:], bias_sb[0:1, :],
                    start=True, stop=False, skip_group_check=True,
                )
                PC = CFG["pool_ci_s"]
                hi = CI - PC  # GPSIMD takes the last-consumed ci-slice
                if PC:
                    nc.gpsimd.tensor_mul(tmp[:, hi:], U[:, hi:], facb[:, hi:])
                AC = CFG["acc_chunk"]
                chunks = [(0, hi // 2), (hi // 2, hi)]
                done = 0
                for q0, q1 in chunks:
                    sl = slice(q0, q1)
                    nc.vector.tensor_mul(tmp[:, sl], U[:, sl], facb[:, sl])
                    yield _dve((q1 - q0) * OUTCH / 2)
                    for c0 in range(q0, q1, AC):
                        cn = min(c0 + AC, q1)
                        for ci in range(c0, cn):
                            nc.tensor.matmul(
                                SB, identb_sb[:],
                                tmp[:, ci].rearrange("p n c -> p (n c)"),
                                start=False, stop=(ci == CI - 1),
                                skip_group_check=True,
                            )
                        yield _pe((cn - c0) * OUTCH, cn - c0)
                for ci in range(hi, CI):
                    nc.tensor.matmul(
                        SB, identb_sb[:],
                        tmp[:, ci].rearrange("p n c -> p (n c)"),
                        start=False, stop=(ci == CI - 1),
                        skip_group_check=True,
                    )
                if PC:
                    yield _pe(PC * OUTCH, PC)
                s_phase.out = SB
                s_phase.out_tile = SBt

            def d_phase_exp(t, U, V, E, Eprev, it, Dh_in=None, Dh_out=None):
                # Fused: tmpn = U*V in no-major layout (strided write keeps
                # co innermost -> DVE 2x survives), pipelined per ci-half
                # into PE accumulating copies D_h[p,(ci_h,co)] = sum_no tmpn.
                # Each half's exp is taken as soon as it finishes, so the
                # 1-bank D tile frees immediately (E co-major for the PE
                # softmax sum; iteration-3 E multiplies in Eprev here).
                tmpn = bigp.tile([128, NO, CI, CO], bf16, tag="tmp",
                                 name=f"tmpd{it}_{t}")
                tmp = tmpn[:].transpose([0, 2, 1, 3])
                facb = V[:].unsqueeze(1).broadcast_to([128, CI, NO, CO])
                mvs = [tmpn[:, no].rearrange("p c o -> p (c o)")
                       for no in range(NO)]
                PC = CFG["pool_ci_d"] if it == 1 else CFG["pool_ci_d2"]
                HN = NO // 2
                for h in range(2):
                    # GPSIMD slice sits in half 1 (consumed last); emit it
                    # during half 0 so it has a full half of slack.
                    if h == 0 and PC:
                        cl = slice(HCI, HCI + PC)
                        nc.gpsimd.tensor_mul(tmp[:, cl], U[:, cl], facb[:, cl])
                    cv = slice(h * HCI + (PC if h == 1 else 0), (h + 1) * HCI)
                    ne = (cv.stop - cv.start) * HN * CO
                    nc.vector.tensor_mul(
                        tmp[:, cv, :HN], U[:, cv, :HN], facb[:, cv, :HN]
                    )
                    yield _merge(_dve(ne / 2), _pool(PC * NO * CO) if h == 0 and PC else {})
                    nc.vector.tensor_mul(
                        tmp[:, cv, HN:], U[:, cv, HN:], facb[:, cv, HN:]
                    )
                    yield _dve(ne / 2)
                    if Dh_in is not None:
                        Dh = Dh_in[h]  # accumulate onto resident D1 half
                    else:
                        Dh = pdp.tile([128, 512], f32, tag="D",
                                      name=f"D{t}_{it}_{h}")
                    if Dh_out is not None:
                        Dh_out.append(Dh)
                    for no in range(HN):
                        nc.tensor.matmul(
                            Dh[:], identb_sb[:],
                            mvs[no][:, h * 512 : (h + 1) * 512],
                            start=(no == 0 and Dh_in is None), stop=False,
                            skip_group_check=True,
                        )
                    yield _pe(HN * 512, HN)
                    for no in range(HN, NO):
                        nc.tensor.matmul(
                            Dh[:], identb_sb[:],
                            mvs[no][:, h * 512 : (h + 1) * 512],
                            start=False,
                            stop=(no == NO - 1 and Dh_out is None),
                            skip_group_check=True,
                        )
                    yield _pe(HN * 512, HN)
                    # exp of this half -> E co-major; frees the D bank
                    hs = slice(h * HCI, (h + 1) * HCI)
                    Lv = Dh[:].rearrange("p (i c) -> p i c", i=HCI)
                    nc.scalar.activation(
                        E[:, :, hs].transpose([0, 2, 1]), Lv, AF.Exp
                    )
                    tag = _act(512)
                    if Eprev is not None:
                        # consumed by next group's tail: long slack -> GPSIMD
                        if CFG["emul_pool"]:
                            nc.gpsimd.tensor_mul(
                                E[:, :, hs], E[:, :, hs], Eprev[:, :, hs]
                            )
                            tag = _merge(tag, _pool(256))
                        else:
                            nc.vector.tensor_mul(
                                E[:, :, hs], E[:, :, hs], Eprev[:, :, hs]
                            )
                            tag = _merge(tag, _dve(256))
                    yield tag

            def d2_dma_tree(t, U, V, E, Eprev):
                # Iteration-2 d-phase with the no-contraction done by SWDGE
                # accumulate-DMAs (3-level in-place tree over tmpn's no axis):
                # no PE, no PSUM; E3 is only consumed by the NEXT group's
                # tail, so the DMA latency rides in pure slack.
                tmpn = bigp.tile([128, NO, CI, CO], bf16, tag="tmp",
                                 name=f"tmpd2_{t}")
                tmp = tmpn[:].transpose([0, 2, 1, 3])
                facb = V[:].unsqueeze(1).broadcast_to([128, CI, NO, CO])
                PC = CFG["pool_ci_d2"]
                if PC:
                    cl = slice(HCI, HCI + PC)
                    nc.gpsimd.tensor_mul(tmp[:, cl], U[:, cl], facb[:, cl])
                nc.vector.tensor_mul(tmp[:, :HCI // 2], U[:, :HCI // 2],
                                     facb[:, :HCI // 2])
                yield _merge(_dve(HCI // 2 * NO * CO / 2),
                             _pool(PC * NO * CO) if PC else {})
                nc.vector.tensor_mul(tmp[:, HCI // 2 : HCI],
                                     U[:, HCI // 2 : HCI],
                                     facb[:, HCI // 2 : HCI])
                yield _dve(HCI // 2 * NO * CO / 2)
                cv = slice(HCI + PC, CI)
                if cv.start < cv.stop:
                    nc.vector.tensor_mul(tmp[:, cv], U[:, cv], facb[:, cv])
                    yield _dve((cv.stop - cv.start) * NO * CO / 2)
                add = OP.add
                nc.gpsimd.dma_start(tmpn[:, 0:4], tmpn[:, 4:8], accum_op=add)
                yield {"pool": 1040, "dma": 2950}
                nc.gpsimd.dma_start(tmpn[:, 0:2], tmpn[:, 2:4], accum_op=add)
                yield {"pool": 1040, "dma": 1500}
                nc.gpsimd.dma_start(tmpn[:, 0:1], tmpn[:, 1:2], accum_op=add)
                yield {"pool": 1040, "dma": 780}
                D2 = tmpn[:, 0]  # [p, ci, co] bf16
                nc.scalar.activation(E[:].transpose([0, 2, 1]), D2[:], AF.Exp)
                if CFG["emul_pool"]:
                    nc.gpsimd.tensor_mul(E[:], E[:], Eprev[:])
                    yield _merge(_act(1024), _pool(1024))
                else:
                    nc.vector.tensor_mul(E[:], E[:], Eprev[:])
                    yield _merge(_act(1024), _dve(512))

            def softmax_r(t, E, it):
                # E stored co-major so the PE does the softmax sum as 8
                # accumulating copies (sum over co-quarters), leaving only
                # a 4-wide DVE reduce; R = E * (1/sum) broadcast.
                Ev = E[:].transpose([0, 2, 1])
                R = statep.tile([128, CI, CO], bf16, tag="R",
                                name=f"R{t}_{it}")
                sume = statep.tile([128, CI], f32, tag="sume",
                                   name=f"sume{t}_{it}")
                rec = statep.tile([128, CI], f32, tag="rec",
                                  name=f"rec{t}_{it}")
                qacc = pqp.tile([128, 128], f32, tag="qa",
                                name=f"qa{t}_{it}")
                Ef = E[:].rearrange("p c i -> p (c i)")
                for b in range(NO):
                    nc.tensor.matmul(
                        qacc[:], identb_sb[:],
                        Ef[:, b * 128 : (b + 1) * 128],
                        start=(b == 0), stop=(b == NO - 1),
                        skip_group_check=True,
                    )
                yield _pe(NO * 128, NO)
                nc.vector.tensor_reduce(
                    sume[:],
                    qacc[:].rearrange("p (q i) -> p q i", q=4)
                    .transpose([0, 2, 1]),
                    axis=AX.X, op=OP.add,
                )
                nc.vector.reciprocal(rec[:], sume[:])
                yield _dve(260, f32=True)
                PC = CFG["pool_ci_r"]
                hi = CI - PC
                recb = rec[:].unsqueeze(2).broadcast_to([128, CI, CO])
                if PC:
                    nc.gpsimd.tensor_mul(R[:, hi:], Ev[:, hi:], recb[:, hi:])
                nc.vector.tensor_mul(R[:, :hi], Ev[:, :hi], recb[:, :hi])
                yield _merge(_dve(hi * CO / 2), _pool(PC * CO) if PC else {})
                softmax_r.out = R

            def main_tile(t):
                U, S1 = conv_tile.out.pop(t)
                # ---- iteration 1: S1 came scaled+biased out of the conv --
                if CFG["sq1_conv"]:
                    V1 = conv_tile.v1.pop(t)
                else:
                    S1v = S1[:].rearrange("p (n c) -> p n c", n=NO)
                    yield from squash(t, S1v, 1, bf16, CFG["newton12"],
                                      Sb=S1v if CFG["s1bf"] else None)
                    V1 = squash.out
                E2 = ep.tile([128, CO, CI], bf16, tag="E2", name=f"E2_{t}")
                D1h = [] if CFG["dsum"] else None
                yield from d_phase_exp(t, U, V1, E2, None, 1, Dh_out=D1h)
                # ---- iteration 2 ----
                yield from softmax_r(t, E2, 2)
                R2 = softmax_r.out
                yield from s_phase(t, U, R2, 2)
                SB = s_phase.out
                Sv = SB.rearrange("p (n c) -> p n c", n=NO)
                yield from squash(t, Sv, 2, bf16, CFG["newton12"])
                V2 = squash.out
                E3 = ep.tile([128, CO, CI], bf16, tag="E3", name=f"E3_{t}")
                if CFG["dsum"]:
                    # accumulate D2 onto the still-resident D1 halves: exp3
                    # reads D1+D2 directly, no E2-multiply needed
                    yield from d_phase_exp(t, U, V2, E3, None, 2, Dh_in=D1h)
                elif CFG["d2_dma"]:
                    yield from d2_dma_tree(t, U, V2, E3, E2)
                else:
                    yield from d_phase_exp(t, U, V2, E3, E2, 2)
                main_tile.state[t] = (U, E3)
            main_tile.state = {}

            def tail_tile(t, gated=False):
                if gated:
                    # spin (cost-model no-ops) until our main has finished
                    # emitting; lets the last tails share their mains' drain
                    while t not in main_tile.state:
                        yield {"nop": 50}
                U, E3 = main_tile.state.pop(t)
                yield from softmax_r(t, E3, 3)
                R3 = softmax_r.out
                yield from s_phase(t, U, R3, 3)
                SB = s_phase.out
                SBt3 = s_phase.out_tile
                Sv = SB.rearrange("p (n c) -> p n c", n=NO)
                yield from squash(t, Sv, 3, f32, CFG["newton3"])
                emit_out(t, squash.out, SBt3)
                yield _merge(_pe(256, 2), _act(256))

            def drain(gens):
                # Greedy list scheduler over generator segments: advance the
                # generator whose next (already-known) segment completes
                # earliest on a virtual per-engine timeline.  A segment that
                # consumes another engine's result pays a semaphore-handoff
                # latency before it can start.
                HO = CFG["handoff"]
                rng = drain.rng
                jit = CFG["jit"] if CFG["seed"] else 0.0

                def jitter(tag):
                    if not jit:
                        return tag
                    return {e: ns * (1 + rng.uniform(-jit, jit))
                            for e, ns in tag.items()}

                vclock = dict(drain.vclock)
                state = []
                for g in gens:
                    try:
                        tag = jitter(next(g))
                        state.append([g, tag, 0.0, frozenset(tag)])
                    except StopIteration:
                        pass
                while state:
                    best, best_end = None, None
                    for st in state:
                        g, tag, t0, prev = st
                        dep = t0 + (HO if not set(tag) <= prev else 0)
                        start = max([dep] + [vclock.get(e, 0.0)
                                             for e in tag])
                        end = start + max(tag.values()) if tag else start
                        if best_end is None or end < best_end:
                            best, best_end = st, end
                    g, tag, t0, prev = best
                    dep = t0 + (HO if not set(tag) <= prev else 0)
                    start = max([dep] + [vclock.get(e, 0.0) for e in tag])
                    for e, ns in tag.items():
                        vclock[e] = max(vclock.get(e, 0.0), start) + ns
                    best[2] = start + (max(tag.values()) if tag else 0.0)
                    best[3] = frozenset(tag)
                    try:
                        best[1] = jitter(next(g))
                    except StopIteration:
                        state.remove(best)
                drain.vclock = vclock
                global _LAST_VCLOCK
                _LAST_VCLOCK = dict(vclock)
            drain.vclock = {}
            import random as _random
            drain.rng = _random.Random(CFG["seed"] or 1)

            # Software pipeline: tail(g-1) || main(g) || conv(g+1)
            groups = CFG.get("groups", [(0, 1), (2, 3), (4, 5), (6, 7)])
            last = len(groups) - 1
            drain([conv_tile(t) for t in groups[0]])
            for gi, grp in enumerate(groups):
                gens = [main_tile(t) for t in grp]
                if gi > 0:
                    gens += [tail_tile(t) for t in groups[gi - 1]]
                if gi + 1 < len(groups):
                    gens += [conv_tile(t) for t in groups[gi + 1]]
                if gi == last and CFG["gated_tail"]:
                    gens += [tail_tile(t, gated=True) for t in grp]
                drain(gens)
            if not CFG["gated_tail"]:
                drain([tail_tile(t) for t in groups[-1]])

    nc.compile()
    _BUILT[key] = nc
    return nc


def _assemble(out_halves_all):
    o = out_halves_all.reshape(-1, 2, 4, CO, NPIX)
    return np.ascontiguousarray(
        o.transpose(0, 3, 1, 2, 4).reshape(-1, CO, NO, H, W)
    )


def kernel(x, conv_w, bias):
    import sys
    if "/opt/trn_rl_repo" not in sys.path:
        sys.path.insert(0, "/opt/trn_rl_repo")
    from concourse import bass_utils

    patches, w_m, bias_bc, ident, identb = _host_prep(x, conv_w, bias)
    nc = _build_nc()
    in_maps = [
        {"patches": patches[b], "w": w_m, "bias": bias_bc, "ident": ident,
         "identb": identb}
        for b in range(BS)
    ]
    res = bass_utils.run_bass_kernel_spmd(nc, in_maps, core_ids=list(range(BS)))
    outs = np.stack([r["out"] for r in res.results])
    return _assemble(outs).astype(np.float32)


# revision 63
# speedup vs baseline: 1.0015x; 1.0015x over previous
"""Trainium2 Bass kernel for ConvPixelToCapsules (conv -> 3-iter dynamic routing).

Strategy (hardcoded for x[8,32,8,32,32], conv_w[256,8,3,3], bias[32,8,1,1]):
  - Host precomputes im2col patches per batch element with two extra tricks:
    a 33rd "channel" slot holding sum_ci(x) (conv linearity gives iteration
    1's uniform-route preactivation for free) and a 73rd contraction row
    (1.0 in the xsum slot only) whose weight row is CI*bias, so iteration
    1's preactivation S1 = Uxs/CI comes out of the conv bias-included as a
    single scaled PSUM evacuation.
  - 8 NeuronCores, data-parallel over batch: core k owns batch element k.
  - Per core: 8 tiles of 128 output pixels, software-pipelined in pairs
    three groups deep: tail(g-1) || main(g) || conv(g+1).  Phases:
      conv(t): patches DMA, conv matmuls, Act PSUM evacuation, squash1
      main(t): d1, softmax2, s2, squash2, d2, exp3 (iterations 1+2)
      tail(t): softmax3 (from E3 in SBUF), s3, squash3, transpose, out DMA
  - Emission order = per-engine program order (queues are in-order), so
    drain() is a greedy list scheduler: each generator segment carries an
    {engine: est_ns} cost tag and the scheduler advances the generator
    whose segment finishes earliest on a virtual per-engine timeline.
  - Votes live in SBUF as [pixel-partition; (ci,no,co)] bf16; all routing
    products are DVE bf16 2x ops with the last-consumed ci-slices offloaded
    to GPSIMD (so the slow engine never gates the PE accumulation).
  - Both routing contractions run on the PE as accumulating identity
    matmuls; the d-contraction runs per ci-half into a 1-bank PSUM tile
    whose exp is taken immediately, freeing the bank (D never persists:
    iteration 3 uses exp(D1+D2) = E2*exp(D2), computed at main-end).
  - Softmax sums over co run on the PE into a shared qacc bank; squash
    computes sqrt via Quake-rsqrt + Newton on DVE bit ops so the Act engine
    only ever needs Copy+Exp (one act-table load for the whole program).
  - PSUM budget (8 banks): conv pv ring 2, D-half ring 3, S ring 2, qacc 1.
"""

import numpy as np

BS, CI, NI, H, W = 8, 32, 8, 32, 32
CO, NO = 32, 8
NPIX = H * W            # 1024
TILES = 8               # tiles of 128 pixels per batch element
TP = 128                # pixels per tile (on partitions)
K = 73                  # ni*3*3 contraction + bias row
SLOTS = CI + 1          # 32 ci + xsum slot
OUTCH = NO * CO         # 256, (no, co) order
QK = 0x5F3759DF         # Quake rsqrt seed constant
HCI = CI // 2           # 16, ci-half for D tiles

CFG = {
    "warmup": 40,
    "newton12": 1,          # Newton iterations for squash 1-2
    "newton3": 1,           # Newton iterations for final squash
    "pool_ci_d": 2,         # ci (of 16, in half 1 only) of d1-products on GPSIMD
    "pool_ci_d2": 8,        # ci (of 16, half 1) of d2-products on GPSIMD
    "pool_ci_s": 8,         # trailing ci (of 32) of s2-products on GPSIMD
    "pool_ci_s3": 9,        # trailing ci (of 32) of s3-products (tail) on GPSIMD
    "pool_ci_r": 14,        # trailing ci (of 32) of R2-product on GPSIMD
    "pool_ci_r3": 15,       # trailing ci (of 32) of R3-product (tail) on GPSIMD
    "evac_pool": 0,         # (GPSIMD cannot access PSUM: must stay 0)
    "evac_dve": 0,          # ... how many on DVE
    "head_evac_dve": 0,     # evacs on DVE for tiles 0-1 (pipeline fill)
    "acc_chunk": 4,         # s-phase accum matmuls per segment
    "red_pool": 0,          # (unsupported: GPSIMD reduce is partition-axis only)
    "emul_pool": 0,         # iteration-3 E2 multiply on GPSIMD
    "s1bf": 1,              # S1 in bf16 (V1-multiply gets DVE 2x)
    "sq_pool": 0,           # squash Quake-rsqrt chain on GPSIMD
    "gated_tail": 0,        # last group's tails share their mains' drain
    "fused_head": 0,        # group-0 mains gated into the first drain
    "sq1_conv": 1,          # emit squash1 inside conv_tile (early Act slot)
    "d2_dma": 0,            # iteration-2 d-contraction as SWDGE accum-DMA tree
    "dsum": 0,              # hold D1 in PSUM; d2 accumulates onto it
    "handoff": 125,         # scheduler estimate of cross-engine sem latency
    "seed": 0,              # scheduler jitter seed (0 = no jitter)
    "jit": 0.15,            # jitter amplitude on segment estimates
    "votes_bufs": 5,
    "big_bufs": 3,
    "pat_bufs": 3,
    "pconv_bufs": 2,
    "pd_bufs": 3,
    "ps_bufs": 2,
}

_BUILT = {}

# segment cost helpers (ns estimates for the emission scheduler)
def _dve(elems, f32=False):
    return {"dve": elems / (0.96 if f32 else 1.92) + 130}

def _pool(elems):
    return {"pool": elems / 0.504 + 150}

def _act(elems):
    return {"act": elems / 1.2 + 220}

def _pe(cols, n=1):
    return {"pe": cols * 0.417 + n * 6}

def _merge(*tags):
    out = {}
    for t in tags:
        for k, v in t.items():
            out[k] = out.get(k, 0.0) + v
    return out


def _host_prep(x, conv_w, bias):
    x = np.asarray(x, np.float32)
    conv_w = np.asarray(conv_w, np.float32)
    bias = np.asarray(bias, np.float32)
    x_pad = np.pad(x, ((0, 0), (0, 0), (0, 0), (1, 1), (1, 1)))
    x_aug = np.concatenate([x_pad, x_pad.sum(1, keepdims=True)], axis=1)
    wv = np.lib.stride_tricks.sliding_window_view(x_aug, (3, 3), axis=(3, 4))
    import ml_dtypes
    cdt_np = ml_dtypes.bfloat16
    patches = np.ascontiguousarray(
        wv.transpose(0, 2, 5, 6, 1, 3, 4).reshape(BS, K - 1, SLOTS, NPIX)
    ).astype(cdt_np)
    # row 72: 1.0 in the xsum slot only -> the conv adds CI*bias to Uxs,
    # making iteration 1's preactivation a pure scaled copy at evac time.
    brow = np.zeros((BS, 1, SLOTS, NPIX), dtype=cdt_np)
    brow[:, :, CI, :] = 1.0
    patches = np.concatenate([patches, brow], axis=1)
    w_m = np.ascontiguousarray(
        conv_w.reshape(CO, NO, NI, 3, 3).transpose(2, 3, 4, 1, 0)
        .reshape(K - 1, OUTCH)
    ).astype(cdt_np)
    w_m = np.concatenate(
        [w_m,
         (CI * bias[:, :, 0, 0].T.reshape(1, OUTCH)).astype(cdt_np)], axis=0)
    bias_bc = np.broadcast_to(
        bias[:, :, 0, 0].T.reshape(1, OUTCH), (128, OUTCH)
    ).astype(np.float32)
    ident = np.eye(128, dtype=np.float32)
    identb = np.eye(128, dtype=cdt_np)
    return patches, w_m, bias_bc, ident, identb


def _build_nc():
    def _freeze(v):
        if isinstance(v, (list, tuple)):
            return tuple(_freeze(x) for x in v)
        return v
    key = ("nc",) + tuple(sorted((k, _freeze(v)) for k, v in CFG.items()))
    if key in _BUILT:
        return _BUILT[key]
    import concourse.bacc as bacc
    import concourse.tile as tile
    import concourse.mybir as mybir

    f32 = mybir.dt.float32
    bf16 = mybir.dt.bfloat16
    u32 = mybir.dt.uint32
    AF = mybir.ActivationFunctionType
    OP = mybir.AluOpType
    AX = mybir.AxisListType

    nc = bacc.Bacc("TRN2", target_bir_lowering=False, debug=False, num_devices=8)

    patches_d = nc.dram_tensor("patches", [K, SLOTS, NPIX], bf16, kind="ExternalInput")
    w_d = nc.dram_tensor("w", [K, OUTCH], bf16, kind="ExternalInput")
    bias_d = nc.dram_tensor("bias", [128, OUTCH], f32, kind="ExternalInput")
    ident_d = nc.dram_tensor("ident", [128, 128], f32, kind="ExternalInput")
    identb_d = nc.dram_tensor("identb", [128, 128], bf16, kind="ExternalInput")
    out_d = nc.dram_tensor("out", [2, 128, NPIX], f32, kind="ExternalOutput")

    with tile.TileContext(nc) as tc:
        with (
            tc.tile_pool(name="const", bufs=1) as const,
            tc.tile_pool(name="pat", bufs=CFG["pat_bufs"]) as patp,
            tc.tile_pool(name="votes", bufs=CFG["votes_bufs"]) as votesp,
            tc.tile_pool(name="s1", bufs=4) as s1p,
            tc.tile_pool(name="big", bufs=CFG["big_bufs"]) as bigp,
            tc.tile_pool(name="state", bufs=4) as statep,
            tc.tile_pool(name="ep", bufs=3) as ep,
            tc.tile_pool(name="obuf", bufs=1) as obufp,
            tc.tile_pool(name="pconv", bufs=CFG["pconv_bufs"], space="PSUM") as pconv,
            tc.tile_pool(name="pd", bufs=CFG["pd_bufs"], space="PSUM") as pdp,
            tc.tile_pool(name="ps", bufs=CFG["ps_bufs"], space="PSUM") as psp,
            tc.tile_pool(name="pq", bufs=1, space="PSUM") as pqp,
        ):
            w_sb = const.tile([K, OUTCH], bf16)
            nc.sync.dma_start(w_sb[:], w_d.ap())
            bias_sb = const.tile([128, OUTCH], f32)
            nc.sync.dma_start(bias_sb[:], bias_d.ap())
            ident_sb = const.tile([128, 128], f32)
            nc.sync.dma_start(ident_sb[:], ident_d.ap())
            identb_sb = const.tile([128, 128], bf16)
            nc.sync.dma_start(identb_sb[:], identb_d.ap())
            ones1 = const.tile([1, 128], f32)
            nc.gpsimd.memset(ones1[:], 1.0)
            qc = const.tile([128, 2 * CO], u32)
            nc.gpsimd.memset(qc[:], QK)

            ob = [
                obufp.tile([128, NPIX], f32, tag=f"ob{h}", name=f"ob{h}")
                for h in range(2)
            ]

            # PE p-state warmup: the tensor engine needs ~3us of continuous
            # work to reach full clock; a burst of dependency-free matmuls
            # fills the initial patches-DMA window.
            warm = pqp.tile([128, 128], f32, tag="qa", name="warm")
            for _ in range(CFG["warmup"]):
                nc.tensor.matmul(
                    warm[:, :64], identb_sb[:], identb_sb[:, :64],
                    start=True, stop=True, skip_group_check=True,
                )

            def conv_tile(t):
                # votes for 128 pixels; Uxs slot first so iteration 1 can
                # start early; ci-pairs share one PSUM bank so evacuation
                # runs as double-width copies.  squash1 runs here (it only
                # needs S1, ready after the first matmul) so its Act square
                # isn't queued behind all 16 in-order evacuations.
                pt = patp.tile([K, SLOTS, TP], bf16, tag="pt", name=f"pt{t}")
                nc.sync.dma_start(
                    pt[:, CI, :], patches_d.ap()[:, CI, t * TP : (t + 1) * TP]
                )
                for dq in range(4):
                    qs = slice(dq * 8, (dq + 1) * 8)
                    nc.sync.dma_start(
                        pt[:, qs, :],
                        patches_d.ap()[:, qs, t * TP : (t + 1) * TP],
                    )
                U = votesp.tile([128, CI, NO, CO], bf16, tag="U", name=f"U{t}")
                S1 = s1p.tile([128, OUTCH], bf16 if CFG["s1bf"] else f32,
                              tag="S1", name=f"S1{t}")
                conv_tile.out[t] = (U, S1)
                pvx = pconv.tile([128, 2 * OUTCH], f32, tag="pv", name=f"pvx{t}")
                nc.tensor.matmul(
                    pvx[:, :OUTCH], pt[:, CI, :], w_sb[:], start=True, stop=True
                )
                nc.scalar.activation(S1[:], pvx[:, :OUTCH], AF.Copy,
                                     0.0, scale=1.0 / CI)
                yield _merge(_pe(256), _act(256))
                if CFG["sq1_conv"]:
                    S1v = S1[:].rearrange("p (n c) -> p n c", n=NO)
                    yield from squash(t, S1v, 1, bf16, CFG["newton12"],
                                      Sb=S1v if CFG["s1bf"] else None)
                    conv_tile.v1[t] = squash.out
                nd, npo = CFG["evac_dve"], CFG["evac_pool"]
                if t < 2:
                    # pipeline fill: DVE is idle during the first convs, so
                    # splitting the evacuation halves the serial evac wall
                    nd = max(nd, CFG["head_evac_dve"])
                for c in range(CI // 2):
                    pv = pconv.tile([128, 2 * OUTCH], f32, tag="pv",
                                    name=f"pv{t}_{c}")
                    nc.tensor.matmul(
                        pv[:, :OUTCH], pt[:, 2 * c, :], w_sb[:],
                        start=True, stop=True,
                    )
                    nc.tensor.matmul(
                        pv[:, OUTCH:], pt[:, 2 * c + 1, :], w_sb[:],
                        start=True, stop=True,
                    )
                    dst = U[:, 2 * c : 2 * c + 2].rearrange(
                        "p c n o -> p (c n o)"
                    )
                    if c < nd:
                        nc.vector.tensor_copy(dst, pv[:])
                        tag = _dve(512, f32=True)
                    elif c < nd + npo:
                        nc.gpsimd.tensor_copy(dst, pv[:])
                        tag = _pool(512)
                    else:
                        nc.scalar.copy(dst, pv[:])
                        tag = _act(512)
                    yield _merge(_pe(512, 2), tag)
            conv_tile.out = {}
            conv_tile.v1 = {}

            def emit_out(t, V, SBt):
                # transposes land in the upper half of the final iteration's
                # S PSUM bank (no separate PSUM pool needed)
                Vf = V[:].rearrange("p n c -> p (n c)")
                for h in range(2):
                    tp = SBt[:, OUTCH + h * 128 : OUTCH + (h + 1) * 128]
                    nc.tensor.transpose(
                        tp, Vf[:, h * 128 : (h + 1) * 128], ident_sb[:]
                    )
                    nc.scalar.copy(ob[h][:, t * TP : (t + 1) * TP], tp)
                    nc.sync.dma_start(
                        out_d.ap()[h][:, t * TP : (t + 1) * TP],
                        ob[h][:, t * TP : (t + 1) * TP],
                    )

            def squash(t, S, it, out_dtype, newton, Sb=None):
                # S: [128, NO, CO] f32 (SBUF or PSUM view) -> V [128, NO, CO]
                # scl = sqrt(n)/(1+n) via Quake rsqrt (no act tables needed).
                # Sb: optional bf16 copy of S (keeps the V-multiply in DVE 2x
                # mode); the norm reduce runs on GPSIMD to spare DVE.
                sq = statep.tile([128, NO, CO], f32, tag="sq", name=f"sq{t}_{it}")
                nc.scalar.square(sq[:], S)
                nsq = statep.tile([128, CO], f32, tag="nsq", name=f"nsq{t}_{it}")
                if CFG["red_pool"]:
                    nc.gpsimd.tensor_reduce(
                        nsq[:], sq[:].transpose([0, 2, 1]), axis=AX.X, op=OP.add
                    )
                    yield _merge(_act(256), _pool(256))
                else:
                    nc.vector.tensor_reduce(
                        nsq[:], sq[:].transpose([0, 2, 1]), axis=AX.X, op=OP.add
                    )
                    yield _merge(_act(256), _dve(256, f32=True))
                eng = nc.gpsimd if CFG["sq_pool"] else nc.vector
                sh = statep.tile([128, CO], u32, tag="sh", name=f"sh{t}_{it}")
                eng.tensor_scalar(
                    sh[:], nsq[:].bitcast(u32), 1, None,
                    op0=OP.logical_shift_right,
                )
                y = statep.tile([128, CO], f32, tag="y", name=f"y{t}_{it}")
                eng.tensor_tensor(
                    y[:].bitcast(u32), qc[:, :CO], sh[:], op=OP.subtract
                )
                den = statep.tile([128, CO], f32, tag="den", name=f"den{t}_{it}")
                eng.tensor_scalar_add(den[:], nsq[:], 1.0)
                rcd = statep.tile([128, CO], f32, tag="rcd", name=f"rcd{t}_{it}")
                nc.vector.reciprocal(rcd[:], den[:])
                tq = statep.tile([128, CO], f32, tag="tq", name=f"tq{t}_{it}")
                for _ in range(newton):
                    eng.tensor_mul(tq[:], y[:], y[:])
                    eng.tensor_mul(tq[:], tq[:], nsq[:])
                    eng.tensor_scalar(
                        tq[:], tq[:], -0.5, 1.5, op0=OP.mult, op1=OP.add
                    )
                    eng.tensor_mul(y[:], y[:], tq[:])
                yield {"pool" if CFG["sq_pool"] else "dve": 900}
                # scl = nsq * y * rcd  (= sqrt(nsq)/(1+nsq))
                sdt = bf16 if Sb is not None else f32
                scl = statep.tile([128, CO], sdt, tag=f"scl{sdt}",
                                  name=f"scl{t}_{it}")
                scm = statep.tile([128, CO], f32, tag="scm", name=f"scm{t}_{it}")
                nc.vector.tensor_mul(scm[:], nsq[:], y[:])
                nc.vector.tensor_mul(scl[:], scm[:], rcd[:])
                V = statep.tile([128, NO, CO], out_dtype, tag=f"V{it}",
                                name=f"V{t}_{it}")
                nc.vector.tensor_mul(
                    V[:], S if Sb is None else Sb,
                    scl[:].unsqueeze(1).broadcast_to([128, NO, CO])
                )
                yield _dve(600, f32=(Sb is None))
                squash.out = V

            def s_phase(t, U, R, it):
                # Fused: tmp = U*R (bf16 2x, trailing ci-slice on GPSIMD)
                # pipelined into the PE ci-contraction. Identity stays the
                # stationary, so each matmul is a PSUM-accumulating copy;
                # bias opens the group as a rank-1 ones x bias_row matmul.
                SBt = psp.tile([128, 512], f32, tag="S", name=f"SB{t}_{it}")
                SB = SBt[:, :OUTCH]
                tmp = bigp.tile([128, CI, NO, CO], bf16, tag="tmp",
                                name=f"tmps{it}_{t}")
                facb = R[:].unsqueeze(2).broadcast_to([128, CI, NO, CO])
                nc.tensor.matmul(
                    SB, ones1[:], bias_sb[0:1, :],
                    start=True, stop=False, skip_group_check=True,
                )
                # iteration 3 runs in the slack-rich tail: bigger GPSIMD share
                PC = CFG["pool_ci_s"] if it == 2 else CFG["pool_ci_s3"]
                hi = CI - PC  # GPSIMD takes the last-consumed ci-slice
                if PC:
                    nc.gpsimd.tensor_mul(tmp[:, hi:], U[:, hi:], facb[:, hi:])
                AC = CFG["acc_chunk"]
                chunks = [(0, hi // 2), (hi // 2, hi)]
                done = 0
                for q0, q1 in chunks:
                    sl = slice(q0, q1)
                    nc.vector.tensor_mul(tmp[:, sl], U[:, sl], facb[:, sl])
                    yield _dve((q1 - q0) * OUTCH / 2)
                    for c0 in range(q0, q1, AC):
                        cn = min(c0 + AC, q1)
                        for ci in range(c0, cn):
                            nc.tensor.matmul(
                                SB, identb_sb[:],
                                tmp[:, ci].rearrange("p n c -> p (n c)"),
                                start=False, stop=(ci == CI - 1),
                                skip_group_check=True,
                            )
                        yield _pe((cn - c0) * OUTCH, cn - c0)
                for ci in range(hi, CI):
                    nc.tensor.matmul(
                        SB, identb_sb[:],
                        tmp[:, ci].rearrange("p n c -> p (n c)"),
                        start=False, stop=(ci == CI - 1),
                        skip_group_check=True,
                    )
                if PC:
                    yield _pe(PC * OUTCH, PC)
                s_phase.out = SB
                s_phase.out_tile = SBt

            def d_phase_exp(t, U, V, E, Eprev, it, Dh_in=None, Dh_out=None):
                # Fused: tmpn = U*V in no-major layout (strided write keeps
                # co innermost -> DVE 2x survives), pipelined per ci-half
                # into PE accumulating copies D_h[p,(ci_h,co)] = sum_no tmpn.
                # Each half's exp is taken as soon as it finishes, so the
                # 1-bank D tile frees immediately (E co-major for the PE
                # softmax sum; iteration-3 E multiplies in Eprev here).
                tmpn = bigp.tile([128, NO, CI, CO], bf16, tag="tmp",
                                 name=f"tmpd{it}_{t}")
                tmp = tmpn[:].transpose([0, 2, 1, 3])
                facb = V[:].unsqueeze(1).broadcast_to([128, CI, NO, CO])
                mvs = [tmpn[:, no].rearrange("p c o -> p (c o)")
                       for no in range(NO)]
                PC = CFG["pool_ci_d"] if it == 1 else CFG["pool_ci_d2"]
                HN = NO // 2
                for h in range(2):
                    # GPSIMD slice sits in half 1 (consumed last); emit it
                    # during half 0 so it has a full half of slack.
                    if h == 0 and PC:
                        cl = slice(HCI, HCI + PC)
                        nc.gpsimd.tensor_mul(tmp[:, cl], U[:, cl], facb[:, cl])
                    cv = slice(h * HCI + (PC if h == 1 else 0), (h + 1) * HCI)
                    ne = (cv.stop - cv.start) * HN * CO
                    nc.vector.tensor_mul(
                        tmp[:, cv, :HN], U[:, cv, :HN], facb[:, cv, :HN]
                    )
                    yield _merge(_dve(ne / 2), _pool(PC * NO * CO) if h == 0 and PC else {})
                    nc.vector.tensor_mul(
                        tmp[:, cv, HN:], U[:, cv, HN:], facb[:, cv, HN:]
                    )
                    yield _dve(ne / 2)
                    if Dh_in is not None:
                        Dh = Dh_in[h]  # accumulate onto resident D1 half
                    else:
                        Dh = pdp.tile([128, 512], f32, tag="D",
                                      name=f"D{t}_{it}_{h}")
                    if Dh_out is not None:
                        Dh_out.append(Dh)
                    for no in range(HN):
                        nc.tensor.matmul(
                            Dh[:], identb_sb[:],
                            mvs[no][:, h * 512 : (h + 1) * 512],
                            start=(no == 0 and Dh_in is None), stop=False,
                            skip_group_check=True,
                        )
                    yield _pe(HN * 512, HN)
                    for no in range(HN, NO):
                        nc.tensor.matmul(
                            Dh[:], identb_sb[:],
                            mvs[no][:, h * 512 : (h + 1) * 512],
                            start=False,
                            stop=(no == NO - 1 and Dh_out is None),
                            skip_group_check=True,
                        )
                    yield _pe(HN * 512, HN)
                    # exp of this half -> E co-major; frees the D bank
                    hs = slice(h * HCI, (h + 1) * HCI)
                    Lv = Dh[:].rearrange("p (i c) -> p i c", i=HCI)
                    nc.scalar.activation(
                        E[:, :, hs].transpose([0, 2, 1]), Lv, AF.Exp
                    )
                    tag = _act(512)
                    if Eprev is not None:
                        # consumed by next group's tail: long slack -> GPSIMD
                        if CFG["emul_pool"]:
                            nc.gpsimd.tensor_mul(
                                E[:, :, hs], E[:, :, hs], Eprev[:, :, hs]
                            )
                            tag = _merge(tag, _pool(256))
                        else:
                            nc.vector.tensor_mul(
                                E[:, :, hs], E[:, :, hs], Eprev[:, :, hs]
                            )
                            tag = _merge(tag, _dve(256))
                    yield tag

            def d2_dma_tree(t, U, V, E, Eprev):
                # Iteration-2 d-phase with the no-contraction done by SWDGE
                # accumulate-DMAs (3-level in-place tree over tmpn's no axis):
                # no PE, no PSUM; E3 is only consumed by the NEXT group's
                # tail, so the DMA latency rides in pure slack.
                tmpn = bigp.tile([128, NO, CI, CO], bf16, tag="tmp",
                                 name=f"tmpd2_{t}")
                tmp = tmpn[:].transpose([0, 2, 1, 3])
                facb = V[:].unsqueeze(1).broadcast_to([128, CI, NO, CO])
                PC = CFG["pool_ci_d2"]
                if PC:
                    cl = slice(HCI, HCI + PC)
                    nc.gpsimd.tensor_mul(tmp[:, cl], U[:, cl], facb[:, cl])
                nc.vector.tensor_mul(tmp[:, :HCI // 2], U[:, :HCI // 2],
                                     facb[:, :HCI // 2])
                yield _merge(_dve(HCI // 2 * NO * CO / 2),
                             _pool(PC * NO * CO) if PC else {})
                nc.vector.tensor_mul(tmp[:, HCI // 2 : HCI],
                                     U[:, HCI // 2 : HCI],
                                     facb[:, HCI // 2 : HCI])
                yield _dve(HCI // 2 * NO * CO / 2)
                cv = slice(HCI + PC, CI)
                if cv.start < cv.stop:
                    nc.vector.tensor_mul(tmp[:, cv], U[:, cv], facb[:, cv])
                    yield _dve((cv.stop - cv.start) * NO * CO / 2)
                add = OP.add
                nc.gpsimd.dma_start(tmpn[:, 0:4], tmpn[:, 4:8], accum_op=add)
                yield {"pool": 1040, "dma": 2950}
                nc.gpsimd.dma_start(tmpn[:, 0:2], tmpn[:, 2:4], accum_op=add)
                yield {"pool": 1040, "dma": 1500}
                nc.gpsimd.dma_start(tmpn[:, 0:1], tmpn[:, 1:2], accum_op=add)
                yield {"pool": 1040, "dma": 780}
                D2 = tmpn[:, 0]  # [p, ci, co] bf16
                nc.scalar.activation(E[:].transpose([0, 2, 1]), D2[:], AF.Exp)
                if CFG["emul_pool"]:
                    nc.gpsimd.tensor_mul(E[:], E[:], Eprev[:])
                    yield _merge(_act(1024), _pool(1024))
                else:
                    nc.vector.tensor_mul(E[:], E[:], Eprev[:])
                    yield _merge(_act(1024), _dve(512))

            def softmax_r(t, E, it):
                # E stored co-major so the PE does the softmax sum as 8
                # accumulating copies (sum over co-quarters), leaving only
                # a 4-wide DVE reduce; R = E * (1/sum) broadcast.
                Ev = E[:].transpose([0, 2, 1])
                R = statep.tile([128, CI, CO], bf16, tag="R",
                                name=f"R{t}_{it}")
                sume = statep.tile([128, CI], f32, tag="sume",
                                   name=f"sume{t}_{it}")
                rec = statep.tile([128, CI], f32, tag="rec",
                                  name=f"rec{t}_{it}")
                qacc = pqp.tile([128, 128], f32, tag="qa",
                                name=f"qa{t}_{it}")
                Ef = E[:].rearrange("p c i -> p (c i)")
                for b in range(NO):
                    nc.tensor.matmul(
                        qacc[:], identb_sb[:],
                        Ef[:, b * 128 : (b + 1) * 128],
                        start=(b == 0), stop=(b == NO - 1),
                        skip_group_check=True,
                    )
                yield _pe(NO * 128, NO)
                nc.vector.tensor_reduce(
                    sume[:],
                    qacc[:].rearrange("p (q i) -> p q i", q=4)
                    .transpose([0, 2, 1]),
                    axis=AX.X, op=OP.add,
                )
                nc.vector.reciprocal(rec[:], sume[:])
                yield _dve(260, f32=True)
                PC = CFG["pool_ci_r"] if it == 2 else CFG["pool_ci_r3"]
                hi = CI - PC
                recb = rec[:].unsqueeze(2).broadcast_to([128, CI, CO])
                if PC:
                    nc.gpsimd.tensor_mul(R[:, hi:], Ev[:, hi:], recb[:, hi:])
                nc.vector.tensor_mul(R[:, :hi], Ev[:, :hi], recb[:, :hi])
                yield _merge(_dve(hi * CO / 2), _pool(PC * CO) if PC else {})
                softmax_r.out = R

            def main_tile(t, gated=False):
                if gated:
                    # head-group fill: spin (cost-model no-ops) until our
                    # conv has produced U/V1, so main(0) overlaps conv(1)
                    need = conv_tile.v1 if CFG["sq1_conv"] else conv_tile.out
                    while t not in need or t not in conv_tile.out:
                        yield {"nop": 50}
                U, S1 = conv_tile.out.pop(t)
                # ---- iteration 1: S1 came scaled+biased out of the conv --
                if CFG["sq1_conv"]:
                    V1 = conv_tile.v1.pop(t)
                else:
                    S1v = S1[:].rearrange("p (n c) -> p n c", n=NO)
                    yield from squash(t, S1v, 1, bf16, CFG["newton12"],
                                      Sb=S1v if CFG["s1bf"] else None)
                    V1 = squash.out
                E2 = ep.tile([128, CO, CI], bf16, tag="E2", name=f"E2_{t}")
                D1h = [] if CFG["dsum"] else None
                yield from d_phase_exp(t, U, V1, E2, None, 1, Dh_out=D1h)
                # ---- iteration 2 ----
                yield from softmax_r(t, E2, 2)
                R2 = softmax_r.out
                yield from s_phase(t, U, R2, 2)
                SB = s_phase.out
                Sv = SB.rearrange("p (n c) -> p n c", n=NO)
                yield from squash(t, Sv, 2, bf16, CFG["newton12"])
                V2 = squash.out
                E3 = ep.tile([128, CO, CI], bf16, tag="E3", name=f"E3_{t}")
                if CFG["dsum"]:
                    # accumulate D2 onto the still-resident D1 halves: exp3
                    # reads D1+D2 directly, no E2-multiply needed
                    yield from d_phase_exp(t, U, V2, E3, None, 2, Dh_in=D1h)
                elif CFG["d2_dma"]:
                    yield from d2_dma_tree(t, U, V2, E3, E2)
                else:
                    yield from d_phase_exp(t, U, V2, E3, E2, 2)
                main_tile.state[t] = (U, E3)
            main_tile.state = {}

            def tail_tile(t, gated=False):
                if gated:
                    # spin (cost-model no-ops) until our main has finished
                    # emitting; lets the last tails share their mains' drain
                    while t not in main_tile.state:
                        yield {"nop": 50}
                U, E3 = main_tile.state.pop(t)
                yield from softmax_r(t, E3, 3)
                R3 = softmax_r.out
                yield from s_phase(t, U, R3, 3)
                SB = s_phase.out
                SBt3 = s_phase.out_tile
                Sv = SB.rearrange("p (n c) -> p n c", n=NO)
                yield from squash(t, Sv, 3, f32, CFG["newton3"])
                emit_out(t, squash.out, SBt3)
                yield _merge(_pe(256, 2), _act(256))

            def drain(gens):
                # Greedy list scheduler over generator segments: advance the
                # generator whose next (already-known) segment completes
                # earliest on a virtual per-engine timeline.  A segment that
                # consumes another engine's result pays a semaphore-handoff
                # latency before it can start.
                HO = CFG["handoff"]
                rng = drain.rng
                jit = CFG["jit"] if CFG["seed"] else 0.0

                def jitter(tag):
                    if not jit:
                        return tag
                    return {e: ns * (1 + rng.uniform(-jit, jit))
                            for e, ns in tag.items()}

                vclock = dict(drain.vclock)
                state = []
                for g in gens:
                    try:
                        tag = jitter(next(g))
                        state.append([g, tag, 0.0, frozenset(tag)])
                    except StopIteration:
                        pass
                while state:
                    best, best_end = None, None
                    for st in state:
                        g, tag, t0, prev = st
                        dep = t0 + (HO if not set(tag) <= prev else 0)
                        start = max([dep] + [vclock.get(e, 0.0)
                                             for e in tag])
                        end = start + max(tag.values()) if tag else start
                        if best_end is None or end < best_end:
                            best, best_end = st, end
                    g, tag, t0, prev = best
                    dep = t0 + (HO if not set(tag) <= prev else 0)
                    start = max([dep] + [vclock.get(e, 0.0) for e in tag])
                    for e, ns in tag.items():
                        vclock[e] = max(vclock.get(e, 0.0), start) + ns
                    best[2] = start + (max(tag.values()) if tag else 0.0)
                    best[3] = frozenset(tag)
                    try:
                        best[1] = jitter(next(g))
                    except StopIteration:
                        state.remove(best)
                drain.vclock = vclock
                global _LAST_VCLOCK
                _LAST_VCLOCK = dict(vclock)
            drain.vclock = {}
            import random as _random
            drain.rng = _random.Random(CFG["seed"] or 1)

            # Software pipeline: tail(g-1) || main(g) || conv(g+1)
            groups = CFG.get("groups", [(0, 1), (2, 3), (4, 5), (6, 7)])
            last = len(groups) - 1
            if CFG["fused_head"]:
                # head fill: mains(0,1) gated into the first drain so the
                # routing of tile 0 overlaps the evacuation of tile 1
                drain([conv_tile(t) for t in groups[0]]
                      + [main_tile(t, gated=True) for t in groups[0]]
                      + [conv_tile(t) for t in groups[1]])
                start_gi = 1
            else:
                drain([conv_tile(t) for t in groups[0]])
                start_gi = 0
            for gi, grp in enumerate(groups):
                if gi < start_gi:
                    continue
                gens = [main_tile(t) for t in grp]
                if gi > 0:
                    gens += [tail_tile(t) for t in groups[gi - 1]]
                if gi + 1 < len(groups):
                    gens += [conv_tile(t) for t in groups[gi + 1]]
                if gi == last and CFG["gated_tail"]:
                    gens += [tail_tile(t, gated=True) for t in grp]
                drain(gens)
            if not CFG["gated_tail"]:
                drain([tail_tile(t) for t in groups[-1]])

    nc.compile()
    _BUILT[key] = nc
    return nc


def _assemble(out_halves_all):
    o = out_halves_all.reshape(-1, 2, 4, CO, NPIX)
    return np.ascontiguousarray(
        o.transpose(0, 3, 1, 2, 4).reshape(-1, CO, NO, H, W)
    )


def kernel(x, conv_w, bias):
    import sys
    if "/opt/trn_rl_repo" not in sys.path:
        sys.path.insert(0, "/opt/trn_rl_repo")
    from concourse import bass_utils

    patches, w_m, bias_bc, ident, identb = _host_prep(x, conv_w, bias)
    nc = _build_nc()
    in_maps = [
        {"patches": patches[b], "w": w_m, "bias": bias_bc, "ident": ident,
         "identb": identb}
        for b in range(BS)
    ]
    res = bass_utils.run_bass_kernel_spmd(nc, in_maps, core_ids=list(range(BS)))
    outs = np.stack([r["out"] for r in res.results])
    return _assemble(outs).astype(np.float32)


# revision 70
# speedup vs baseline: 1.0067x; 1.0052x over previous
"""Trainium2 Bass kernel for ConvPixelToCapsules (conv -> 3-iter dynamic routing).

Strategy (hardcoded for x[8,32,8,32,32], conv_w[256,8,3,3], bias[32,8,1,1]):
  - Host precomputes im2col patches per batch element with two extra tricks:
    a 33rd "channel" slot holding sum_ci(x) (conv linearity gives iteration
    1's uniform-route preactivation for free) and a 73rd contraction row
    (1.0 in the xsum slot only) whose weight row is CI*bias, so iteration
    1's preactivation S1 = Uxs/CI comes out of the conv bias-included as a
    single scaled PSUM evacuation.
  - 8 NeuronCores, data-parallel over batch: core k owns batch element k.
  - Per core: 8 tiles of 128 output pixels, software-pipelined in pairs
    three groups deep: tail(g-1) || main(g) || conv(g+1).  Phases:
      conv(t): patches DMA, conv matmuls, Act PSUM evacuation, squash1
      main(t): d1, softmax2, s2, squash2, d2, exp3 (iterations 1+2)
      tail(t): softmax3 (from E3 in SBUF), s3, squash3, transpose, out DMA
  - Emission order = per-engine program order (queues are in-order), so
    drain() is a greedy list scheduler: each generator segment carries an
    {engine: est_ns} cost tag and the scheduler advances the generator
    whose segment finishes earliest on a virtual per-engine timeline.
  - Votes live in SBUF as [pixel-partition; (ci,no,co)] bf16; all routing
    products are DVE bf16 2x ops with the last-consumed ci-slices offloaded
    to GPSIMD (so the slow engine never gates the PE accumulation).
  - Both routing contractions run on the PE as accumulating identity
    matmuls; the d-contraction runs per ci-half into a 1-bank PSUM tile
    whose exp is taken immediately, freeing the bank (D never persists:
    iteration 3 uses exp(D1+D2) = E2*exp(D2), computed at main-end).
  - Softmax sums over co run on the PE into a shared qacc bank; squash
    computes sqrt via Quake-rsqrt + Newton on DVE bit ops so the Act engine
    only ever needs Copy+Exp (one act-table load for the whole program).
  - PSUM budget (8 banks): conv pv ring 2, D-half ring 3, S ring 2, qacc 1.
"""

import numpy as np

BS, CI, NI, H, W = 8, 32, 8, 32, 32
CO, NO = 32, 8
NPIX = H * W            # 1024
TILES = 8               # tiles of 128 pixels per batch element
TP = 128                # pixels per tile (on partitions)
K = 73                  # ni*3*3 contraction + bias row
SLOTS = CI + 1          # 32 ci + xsum slot
OUTCH = NO * CO         # 256, (no, co) order
QK = 0x5F3759DF         # Quake rsqrt seed constant
HCI = CI // 2           # 16, ci-half for D tiles

CFG = {
    "warmup": 40,
    "newton12": 1,          # Newton iterations for squash 1-2
    "newton3": 1,           # Newton iterations for final squash
    "pool_ci_d": 2,         # ci (of 16, in half 1 only) of d1-products on GPSIMD
    "pool_ci_d2": 8,        # ci (of 16, half 1) of d2-products on GPSIMD
    "pool_ci_s": 8,         # trailing ci (of 32) of s2-products on GPSIMD
    "pool_ci_s3": 9,        # trailing ci (of 32) of s3-products (tail) on GPSIMD
    "pool_ci_r": 14,        # trailing ci (of 32) of R2-product on GPSIMD
    "pool_ci_r3": 15,       # trailing ci (of 32) of R3-product (tail) on GPSIMD
    "pool_ci_s3_last": 8,   # s3 GPSIMD share for the final two tiles (drain)
    "pool_ci_r3_last": 15,   # R3 GPSIMD share for the final two tiles (drain)
    "evac_pool": 0,         # (GPSIMD cannot access PSUM: must stay 0)
    "evac_dve": 0,          # ... how many on DVE
    "head_evac_dve": 0,     # evacs on DVE for tiles 0-1 (pipeline fill)
    "acc_chunk": 4,         # s-phase accum matmuls per segment
    "s_chunks": 2,          # DVE product chunks per s-phase
    "red_pool": 0,          # (unsupported: GPSIMD reduce is partition-axis only)
    "emul_pool": 0,         # iteration-3 E2 multiply on GPSIMD
    "s1bf": 1,              # S1 in bf16 (V1-multiply gets DVE 2x)
    "sq_pool": 0,           # squash Quake-rsqrt chain on GPSIMD
    "gated_tail": 0,        # last group's tails share their mains' drain
    "fused_head": 0,        # group-0 mains gated into the first drain
    "sq1_conv": 1,          # emit squash1 inside conv_tile (early Act slot)
    "d2_dma": 0,            # iteration-2 d-contraction as SWDGE accum-DMA tree
    "dsum": 0,              # hold D1 in PSUM; d2 accumulates onto it
    "handoff": 125,         # scheduler estimate of cross-engine sem latency
    "seed": 0,              # scheduler jitter seed (0 = no jitter)
    "jit": 0.15,            # jitter amplitude on segment estimates
    "votes_bufs": 5,
    "big_bufs": 3,
    "pat_bufs": 3,
    "pconv_bufs": 2,
    "pd_bufs": 3,
    "ps_bufs": 2,
}

_BUILT = {}

# segment cost helpers (ns estimates for the emission scheduler)
def _dve(elems, f32=False):
    return {"dve": elems / (0.96 if f32 else 1.92) + 130}

def _pool(elems):
    return {"pool": elems / 0.504 + 150}

def _act(elems):
    return {"act": elems / 1.2 + 220}

def _pe(cols, n=1):
    return {"pe": cols * 0.417 + n * 6}

def _merge(*tags):
    out = {}
    for t in tags:
        for k, v in t.items():
            out[k] = out.get(k, 0.0) + v
    return out


def _host_prep(x, conv_w, bias):
    x = np.asarray(x, np.float32)
    conv_w = np.asarray(conv_w, np.float32)
    bias = np.asarray(bias, np.float32)
    x_pad = np.pad(x, ((0, 0), (0, 0), (0, 0), (1, 1), (1, 1)))
    x_aug = np.concatenate([x_pad, x_pad.sum(1, keepdims=True)], axis=1)
    wv = np.lib.stride_tricks.sliding_window_view(x_aug, (3, 3), axis=(3, 4))
    import ml_dtypes
    cdt_np = ml_dtypes.bfloat16
    patches = np.ascontiguousarray(
        wv.transpose(0, 2, 5, 6, 1, 3, 4).reshape(BS, K - 1, SLOTS, NPIX)
    ).astype(cdt_np)
    # row 72: 1.0 in the xsum slot only -> the conv adds CI*bias to Uxs,
    # making iteration 1's preactivation a pure scaled copy at evac time.
    brow = np.zeros((BS, 1, SLOTS, NPIX), dtype=cdt_np)
    brow[:, :, CI, :] = 1.0
    patches = np.concatenate([patches, brow], axis=1)
    w_m = np.ascontiguousarray(
        conv_w.reshape(CO, NO, NI, 3, 3).transpose(2, 3, 4, 1, 0)
        .reshape(K - 1, OUTCH)
    ).astype(cdt_np)
    w_m = np.concatenate(
        [w_m,
         (CI * bias[:, :, 0, 0].T.reshape(1, OUTCH)).astype(cdt_np)], axis=0)
    bias_bc = np.broadcast_to(
        bias[:, :, 0, 0].T.reshape(1, OUTCH), (128, OUTCH)
    ).astype(np.float32)
    ident = np.eye(128, dtype=np.float32)
    identb = np.eye(128, dtype=cdt_np)
    return patches, w_m, bias_bc, ident, identb


def _build_nc():
    def _freeze(v):
        if isinstance(v, (list, tuple)):
            return tuple(_freeze(x) for x in v)
        return v
    key = ("nc",) + tuple(sorted((k, _freeze(v)) for k, v in CFG.items()))
    if key in _BUILT:
        return _BUILT[key]
    import concourse.bacc as bacc
    import concourse.tile as tile
    import concourse.mybir as mybir

    f32 = mybir.dt.float32
    bf16 = mybir.dt.bfloat16
    u32 = mybir.dt.uint32
    AF = mybir.ActivationFunctionType
    OP = mybir.AluOpType
    AX = mybir.AxisListType

    nc = bacc.Bacc("TRN2", target_bir_lowering=False, debug=False, num_devices=8)

    patches_d = nc.dram_tensor("patches", [K, SLOTS, NPIX], bf16, kind="ExternalInput")
    w_d = nc.dram_tensor("w", [K, OUTCH], bf16, kind="ExternalInput")
    bias_d = nc.dram_tensor("bias", [128, OUTCH], f32, kind="ExternalInput")
    ident_d = nc.dram_tensor("ident", [128, 128], f32, kind="ExternalInput")
    identb_d = nc.dram_tensor("identb", [128, 128], bf16, kind="ExternalInput")
    out_d = nc.dram_tensor("out", [2, 128, NPIX], f32, kind="ExternalOutput")

    with tile.TileContext(nc) as tc:
        with (
            tc.tile_pool(name="const", bufs=1) as const,
            tc.tile_pool(name="pat", bufs=CFG["pat_bufs"]) as patp,
            tc.tile_pool(name="votes", bufs=CFG["votes_bufs"]) as votesp,
            tc.tile_pool(name="s1", bufs=4) as s1p,
            tc.tile_pool(name="big", bufs=CFG["big_bufs"]) as bigp,
            tc.tile_pool(name="state", bufs=4) as statep,
            tc.tile_pool(name="ep", bufs=3) as ep,
            tc.tile_pool(name="obuf", bufs=1) as obufp,
            tc.tile_pool(name="pconv", bufs=CFG["pconv_bufs"], space="PSUM") as pconv,
            tc.tile_pool(name="pd", bufs=CFG["pd_bufs"], space="PSUM") as pdp,
            tc.tile_pool(name="ps", bufs=CFG["ps_bufs"], space="PSUM") as psp,
            tc.tile_pool(name="pq", bufs=1, space="PSUM") as pqp,
        ):
            w_sb = const.tile([K, OUTCH], bf16)
            nc.sync.dma_start(w_sb[:], w_d.ap())
            bias_sb = const.tile([128, OUTCH], f32)
            nc.sync.dma_start(bias_sb[:], bias_d.ap())
            ident_sb = const.tile([128, 128], f32)
            nc.sync.dma_start(ident_sb[:], ident_d.ap())
            identb_sb = const.tile([128, 128], bf16)
            nc.sync.dma_start(identb_sb[:], identb_d.ap())
            ones1 = const.tile([1, 128], f32)
            nc.gpsimd.memset(ones1[:], 1.0)
            qc = const.tile([128, 2 * CO], u32)
            nc.gpsimd.memset(qc[:], QK)

            ob = [
                obufp.tile([128, NPIX], f32, tag=f"ob{h}", name=f"ob{h}")
                for h in range(2)
            ]

            # PE p-state warmup: the tensor engine needs ~3us of continuous
            # work to reach full clock; a burst of dependency-free matmuls
            # fills the initial patches-DMA window.
            warm = pqp.tile([128, 128], f32, tag="qa", name="warm")
            for _ in range(CFG["warmup"]):
                nc.tensor.matmul(
                    warm[:, :64], identb_sb[:], identb_sb[:, :64],
                    start=True, stop=True, skip_group_check=True,
                )

            def conv_tile(t):
                # votes for 128 pixels; Uxs slot first so iteration 1 can
                # start early; ci-pairs share one PSUM bank so evacuation
                # runs as double-width copies.  squash1 runs here (it only
                # needs S1, ready after the first matmul) so its Act square
                # isn't queued behind all 16 in-order evacuations.
                pt = patp.tile([K, SLOTS, TP], bf16, tag="pt", name=f"pt{t}")
                nc.sync.dma_start(
                    pt[:, CI, :], patches_d.ap()[:, CI, t * TP : (t + 1) * TP]
                )
                for dq in range(4):
                    qs = slice(dq * 8, (dq + 1) * 8)
                    nc.sync.dma_start(
                        pt[:, qs, :],
                        patches_d.ap()[:, qs, t * TP : (t + 1) * TP],
                    )
                U = votesp.tile([128, CI, NO, CO], bf16, tag="U", name=f"U{t}")
                S1 = s1p.tile([128, OUTCH], bf16 if CFG["s1bf"] else f32,
                              tag="S1", name=f"S1{t}")
                conv_tile.out[t] = (U, S1)
                pvx = pconv.tile([128, 2 * OUTCH], f32, tag="pv", name=f"pvx{t}")
                nc.tensor.matmul(
                    pvx[:, :OUTCH], pt[:, CI, :], w_sb[:], start=True, stop=True
                )
                nc.scalar.activation(S1[:], pvx[:, :OUTCH], AF.Copy,
                                     0.0, scale=1.0 / CI)
                yield _merge(_pe(256), _act(256))
                if CFG["sq1_conv"]:
                    S1v = S1[:].rearrange("p (n c) -> p n c", n=NO)
                    yield from squash(t, S1v, 1, bf16, CFG["newton12"],
                                      Sb=S1v if CFG["s1bf"] else None)
                    conv_tile.v1[t] = squash.out
                nd, npo = CFG["evac_dve"], CFG["evac_pool"]
                if t < 2:
                    # pipeline fill: DVE is idle during the first convs, so
                    # splitting the evacuation halves the serial evac wall
                    nd = max(nd, CFG["head_evac_dve"])
                for c in range(CI // 2):
                    pv = pconv.tile([128, 2 * OUTCH], f32, tag="pv",
                                    name=f"pv{t}_{c}")
                    nc.tensor.matmul(
                        pv[:, :OUTCH], pt[:, 2 * c, :], w_sb[:],
                        start=True, stop=True,
                    )
                    nc.tensor.matmul(
                        pv[:, OUTCH:], pt[:, 2 * c + 1, :], w_sb[:],
                        start=True, stop=True,
                    )
                    dst = U[:, 2 * c : 2 * c + 2].rearrange(
                        "p c n o -> p (c n o)"
                    )
                    if c < nd:
                        nc.vector.tensor_copy(dst, pv[:])
                        tag = _dve(512, f32=True)
                    elif c < nd + npo:
                        nc.gpsimd.tensor_copy(dst, pv[:])
                        tag = _pool(512)
                    else:
                        nc.scalar.copy(dst, pv[:])
                        tag = _act(512)
                    yield _merge(_pe(512, 2), tag)
            conv_tile.out = {}
            conv_tile.v1 = {}

            def emit_out(t, V, SBt):
                # transposes land in the upper half of the final iteration's
                # S PSUM bank (no separate PSUM pool needed)
                Vf = V[:].rearrange("p n c -> p (n c)")
                for h in range(2):
                    tp = SBt[:, OUTCH + h * 128 : OUTCH + (h + 1) * 128]
                    nc.tensor.transpose(
                        tp, Vf[:, h * 128 : (h + 1) * 128], ident_sb[:]
                    )
                    nc.scalar.copy(ob[h][:, t * TP : (t + 1) * TP], tp)
                    nc.sync.dma_start(
                        out_d.ap()[h][:, t * TP : (t + 1) * TP],
                        ob[h][:, t * TP : (t + 1) * TP],
                    )

            def squash(t, S, it, out_dtype, newton, Sb=None):
                # S: [128, NO, CO] f32 (SBUF or PSUM view) -> V [128, NO, CO]
                # scl = sqrt(n)/(1+n) via Quake rsqrt (no act tables needed).
                # Sb: optional bf16 copy of S (keeps the V-multiply in DVE 2x
                # mode); the norm reduce runs on GPSIMD to spare DVE.
                sq = statep.tile([128, NO, CO], f32, tag="sq", name=f"sq{t}_{it}")
                nc.scalar.square(sq[:], S)
                nsq = statep.tile([128, CO], f32, tag="nsq", name=f"nsq{t}_{it}")
                if CFG["red_pool"]:
                    nc.gpsimd.tensor_reduce(
                        nsq[:], sq[:].transpose([0, 2, 1]), axis=AX.X, op=OP.add
                    )
                    yield _merge(_act(256), _pool(256))
                else:
                    nc.vector.tensor_reduce(
                        nsq[:], sq[:].transpose([0, 2, 1]), axis=AX.X, op=OP.add
                    )
                    yield _merge(_act(256), _dve(256, f32=True))
                eng = nc.gpsimd if CFG["sq_pool"] else nc.vector
                sh = statep.tile([128, CO], u32, tag="sh", name=f"sh{t}_{it}")
                eng.tensor_scalar(
                    sh[:], nsq[:].bitcast(u32), 1, None,
                    op0=OP.logical_shift_right,
                )
                y = statep.tile([128, CO], f32, tag="y", name=f"y{t}_{it}")
                eng.tensor_tensor(
                    y[:].bitcast(u32), qc[:, :CO], sh[:], op=OP.subtract
                )
                den = statep.tile([128, CO], f32, tag="den", name=f"den{t}_{it}")
                eng.tensor_scalar_add(den[:], nsq[:], 1.0)
                rcd = statep.tile([128, CO], f32, tag="rcd", name=f"rcd{t}_{it}")
                nc.vector.reciprocal(rcd[:], den[:])
                tq = statep.tile([128, CO], f32, tag="tq", name=f"tq{t}_{it}")
                for _ in range(newton):
                    eng.tensor_mul(tq[:], y[:], y[:])
                    eng.tensor_mul(tq[:], tq[:], nsq[:])
                    eng.tensor_scalar(
                        tq[:], tq[:], -0.5, 1.5, op0=OP.mult, op1=OP.add
                    )
                    eng.tensor_mul(y[:], y[:], tq[:])
                yield {"pool" if CFG["sq_pool"] else "dve": 900}
                # scl = nsq * y * rcd  (= sqrt(nsq)/(1+nsq))
                sdt = bf16 if Sb is not None else f32
                scl = statep.tile([128, CO], sdt, tag=f"scl{sdt}",
                                  name=f"scl{t}_{it}")
                scm = statep.tile([128, CO], f32, tag="scm", name=f"scm{t}_{it}")
                nc.vector.tensor_mul(scm[:], nsq[:], y[:])
                nc.vector.tensor_mul(scl[:], scm[:], rcd[:])
                V = statep.tile([128, NO, CO], out_dtype, tag=f"V{it}",
                                name=f"V{t}_{it}")
                nc.vector.tensor_mul(
                    V[:], S if Sb is None else Sb,
                    scl[:].unsqueeze(1).broadcast_to([128, NO, CO])
                )
                yield _dve(600, f32=(Sb is None))
                squash.out = V

            def s_phase(t, U, R, it):
                # Fused: tmp = U*R (bf16 2x, trailing ci-slice on GPSIMD)
                # pipelined into the PE ci-contraction. Identity stays the
                # stationary, so each matmul is a PSUM-accumulating copy;
                # bias opens the group as a rank-1 ones x bias_row matmul.
                SBt = psp.tile([128, 512], f32, tag="S", name=f"SB{t}_{it}")
                SB = SBt[:, :OUTCH]
                tmp = bigp.tile([128, CI, NO, CO], bf16, tag="tmp",
                                name=f"tmps{it}_{t}")
                facb = R[:].unsqueeze(2).broadcast_to([128, CI, NO, CO])
                nc.tensor.matmul(
                    SB, ones1[:], bias_sb[0:1, :],
                    start=True, stop=False, skip_group_check=True,
                )
                # iteration 3 runs in the slack-rich tail: bigger GPSIMD
                # share -- except the final tiles, whose tail is the drain
                # critical path (no next group to hide Pool latency behind)
                if it == 2:
                    PC = CFG["pool_ci_s"]
                elif t >= TILES - 2:
                    PC = CFG["pool_ci_s3_last"]
                else:
                    PC = CFG["pool_ci_s3"]
                hi = CI - PC  # GPSIMD takes the last-consumed ci-slice
                if PC:
                    nc.gpsimd.tensor_mul(tmp[:, hi:], U[:, hi:], facb[:, hi:])
                AC = CFG["acc_chunk"]
                NCH = CFG["s_chunks"]
                bnd = [hi * i // NCH for i in range(NCH + 1)]
                chunks = list(zip(bnd[:-1], bnd[1:]))
                done = 0
                for q0, q1 in chunks:
                    sl = slice(q0, q1)
                    nc.vector.tensor_mul(tmp[:, sl], U[:, sl], facb[:, sl])
                    yield _dve((q1 - q0) * OUTCH / 2)
                    for c0 in range(q0, q1, AC):
                        cn = min(c0 + AC, q1)
                        for ci in range(c0, cn):
                            nc.tensor.matmul(
                                SB, identb_sb[:],
                                tmp[:, ci].rearrange("p n c -> p (n c)"),
                                start=False, stop=(ci == CI - 1),
                                skip_group_check=True,
                            )
                        yield _pe((cn - c0) * OUTCH, cn - c0)
                for ci in range(hi, CI):
                    nc.tensor.matmul(
                        SB, identb_sb[:],
                        tmp[:, ci].rearrange("p n c -> p (n c)"),
                        start=False, stop=(ci == CI - 1),
                        skip_group_check=True,
                    )
                if PC:
                    yield _pe(PC * OUTCH, PC)
                s_phase.out = SB
                s_phase.out_tile = SBt

            def d_phase_exp(t, U, V, E, Eprev, it, Dh_in=None, Dh_out=None):
                # Fused: tmpn = U*V in no-major layout (strided write keeps
                # co innermost -> DVE 2x survives), pipelined per ci-half
                # into PE accumulating copies D_h[p,(ci_h,co)] = sum_no tmpn.
                # Each half's exp is taken as soon as it finishes, so the
                # 1-bank D tile frees immediately (E co-major for the PE
                # softmax sum; iteration-3 E multiplies in Eprev here).
                tmpn = bigp.tile([128, NO, CI, CO], bf16, tag="tmp",
                                 name=f"tmpd{it}_{t}")
                tmp = tmpn[:].transpose([0, 2, 1, 3])
                facb = V[:].unsqueeze(1).broadcast_to([128, CI, NO, CO])
                mvs = [tmpn[:, no].rearrange("p c o -> p (c o)")
                       for no in range(NO)]
                PC = CFG["pool_ci_d"] if it == 1 else CFG["pool_ci_d2"]
                HN = NO // 2
                for h in range(2):
                    # GPSIMD slice sits in half 1 (consumed last); emit it
                    # during half 0 so it has a full half of slack.
                    if h == 0 and PC:
                        cl = slice(HCI, HCI + PC)
                        nc.gpsimd.tensor_mul(tmp[:, cl], U[:, cl], facb[:, cl])
                    cv = slice(h * HCI + (PC if h == 1 else 0), (h + 1) * HCI)
                    ne = (cv.stop - cv.start) * HN * CO
                    nc.vector.tensor_mul(
                        tmp[:, cv, :HN], U[:, cv, :HN], facb[:, cv, :HN]
                    )
                    yield _merge(_dve(ne / 2), _pool(PC * NO * CO) if h == 0 and PC else {})
                    nc.vector.tensor_mul(
                        tmp[:, cv, HN:], U[:, cv, HN:], facb[:, cv, HN:]
                    )
                    yield _dve(ne / 2)
                    if Dh_in is not None:
                        Dh = Dh_in[h]  # accumulate onto resident D1 half
                    else:
                        Dh = pdp.tile([128, 512], f32, tag="D",
                                      name=f"D{t}_{it}_{h}")
                    if Dh_out is not None:
                        Dh_out.append(Dh)
                    for no in range(HN):
                        nc.tensor.matmul(
                            Dh[:], identb_sb[:],
                            mvs[no][:, h * 512 : (h + 1) * 512],
                            start=(no == 0 and Dh_in is None), stop=False,
                            skip_group_check=True,
                        )
                    yield _pe(HN * 512, HN)
                    for no in range(HN, NO):
                        nc.tensor.matmul(
                            Dh[:], identb_sb[:],
                            mvs[no][:, h * 512 : (h + 1) * 512],
                            start=False,
                            stop=(no == NO - 1 and Dh_out is None),
                            skip_group_check=True,
                        )
                    yield _pe(HN * 512, HN)
                    # exp of this half -> E co-major; frees the D bank
                    hs = slice(h * HCI, (h + 1) * HCI)
                    Lv = Dh[:].rearrange("p (i c) -> p i c", i=HCI)
                    nc.scalar.activation(
                        E[:, :, hs].transpose([0, 2, 1]), Lv, AF.Exp
                    )
                    tag = _act(512)
                    if Eprev is not None:
                        # consumed by next group's tail: long slack -> GPSIMD
                        if CFG["emul_pool"]:
                            nc.gpsimd.tensor_mul(
                                E[:, :, hs], E[:, :, hs], Eprev[:, :, hs]
                            )
                            tag = _merge(tag, _pool(256))
                        else:
                            nc.vector.tensor_mul(
                                E[:, :, hs], E[:, :, hs], Eprev[:, :, hs]
                            )
                            tag = _merge(tag, _dve(256))
                    yield tag

            def d2_dma_tree(t, U, V, E, Eprev):
                # Iteration-2 d-phase with the no-contraction done by SWDGE
                # accumulate-DMAs (3-level in-place tree over tmpn's no axis):
                # no PE, no PSUM; E3 is only consumed by the NEXT group's
                # tail, so the DMA latency rides in pure slack.
                tmpn = bigp.tile([128, NO, CI, CO], bf16, tag="tmp",
                                 name=f"tmpd2_{t}")
                tmp = tmpn[:].transpose([0, 2, 1, 3])
                facb = V[:].unsqueeze(1).broadcast_to([128, CI, NO, CO])
                PC = CFG["pool_ci_d2"]
                if PC:
                    cl = slice(HCI, HCI + PC)
                    nc.gpsimd.tensor_mul(tmp[:, cl], U[:, cl], facb[:, cl])
                nc.vector.tensor_mul(tmp[:, :HCI // 2], U[:, :HCI // 2],
                                     facb[:, :HCI // 2])
                yield _merge(_dve(HCI // 2 * NO * CO / 2),
                             _pool(PC * NO * CO) if PC else {})
                nc.vector.tensor_mul(tmp[:, HCI // 2 : HCI],
                                     U[:, HCI // 2 : HCI],
                                     facb[:, HCI // 2 : HCI])
                yield _dve(HCI // 2 * NO * CO / 2)
                cv = slice(HCI + PC, CI)
                if cv.start < cv.stop:
                    nc.vector.tensor_mul(tmp[:, cv], U[:, cv], facb[:, cv])
                    yield _dve((cv.stop - cv.start) * NO * CO / 2)
                add = OP.add
                nc.gpsimd.dma_start(tmpn[:, 0:4], tmpn[:, 4:8], accum_op=add)
                yield {"pool": 1040, "dma": 2950}
                nc.gpsimd.dma_start(tmpn[:, 0:2], tmpn[:, 2:4], accum_op=add)
                yield {"pool": 1040, "dma": 1500}
                nc.gpsimd.dma_start(tmpn[:, 0:1], tmpn[:, 1:2], accum_op=add)
                yield {"pool": 1040, "dma": 780}
                D2 = tmpn[:, 0]  # [p, ci, co] bf16
                nc.scalar.activation(E[:].transpose([0, 2, 1]), D2[:], AF.Exp)
                if CFG["emul_pool"]:
                    nc.gpsimd.tensor_mul(E[:], E[:], Eprev[:])
                    yield _merge(_act(1024), _pool(1024))
                else:
                    nc.vector.tensor_mul(E[:], E[:], Eprev[:])
                    yield _merge(_act(1024), _dve(512))

            def softmax_r(t, E, it):
                # E stored co-major so the PE does the softmax sum as 8
                # accumulating copies (sum over co-quarters), leaving only
                # a 4-wide DVE reduce; R = E * (1/sum) broadcast.
                Ev = E[:].transpose([0, 2, 1])
                R = statep.tile([128, CI, CO], bf16, tag="R",
                                name=f"R{t}_{it}")
                sume = statep.tile([128, CI], f32, tag="sume",
                                   name=f"sume{t}_{it}")
                rec = statep.tile([128, CI], f32, tag="rec",
                                  name=f"rec{t}_{it}")
                qacc = pqp.tile([128, 128], f32, tag="qa",
                                name=f"qa{t}_{it}")
                Ef = E[:].rearrange("p c i -> p (c i)")
                for b in range(NO):
                    nc.tensor.matmul(
                        qacc[:], identb_sb[:],
                        Ef[:, b * 128 : (b + 1) * 128],
                        start=(b == 0), stop=(b == NO - 1),
                        skip_group_check=True,
                    )
                yield _pe(NO * 128, NO)
                nc.vector.tensor_reduce(
                    sume[:],
                    qacc[:].rearrange("p (q i) -> p q i", q=4)
                    .transpose([0, 2, 1]),
                    axis=AX.X, op=OP.add,
                )
                nc.vector.reciprocal(rec[:], sume[:])
                yield _dve(260, f32=True)
                if it == 2:
                    PC = CFG["pool_ci_r"]
                elif t >= TILES - 2:
                    PC = CFG["pool_ci_r3_last"]
                else:
                    PC = CFG["pool_ci_r3"]
                hi = CI - PC
                recb = rec[:].unsqueeze(2).broadcast_to([128, CI, CO])
                if PC:
                    nc.gpsimd.tensor_mul(R[:, hi:], Ev[:, hi:], recb[:, hi:])
                nc.vector.tensor_mul(R[:, :hi], Ev[:, :hi], recb[:, :hi])
                yield _merge(_dve(hi * CO / 2), _pool(PC * CO) if PC else {})
                softmax_r.out = R

            def main_tile(t, gated=False):
                if gated:
                    # head-group fill: spin (cost-model no-ops) until our
                    # conv has produced U/V1, so main(0) overlaps conv(1)
                    need = conv_tile.v1 if CFG["sq1_conv"] else conv_tile.out
                    while t not in need or t not in conv_tile.out:
                        yield {"nop": 50}
                U, S1 = conv_tile.out.pop(t)
                # ---- iteration 1: S1 came scaled+biased out of the conv --
                if CFG["sq1_conv"]:
                    V1 = conv_tile.v1.pop(t)
                else:
                    S1v = S1[:].rearrange("p (n c) -> p n c", n=NO)
                    yield from squash(t, S1v, 1, bf16, CFG["newton12"],
                                      Sb=S1v if CFG["s1bf"] else None)
                    V1 = squash.out
                E2 = ep.tile([128, CO, CI], bf16, tag="E2", name=f"E2_{t}")
                D1h = [] if CFG["dsum"] else None
                yield from d_phase_exp(t, U, V1, E2, None, 1, Dh_out=D1h)
                # ---- iteration 2 ----
                yield from softmax_r(t, E2, 2)
                R2 = softmax_r.out
                yield from s_phase(t, U, R2, 2)
                SB = s_phase.out
                Sv = SB.rearrange("p (n c) -> p n c", n=NO)
                yield from squash(t, Sv, 2, bf16, CFG["newton12"])
                V2 = squash.out
                E3 = ep.tile([128, CO, CI], bf16, tag="E3", name=f"E3_{t}")
                if CFG["dsum"]:
                    # accumulate D2 onto the still-resident D1 halves: exp3
                    # reads D1+D2 directly, no E2-multiply needed
                    yield from d_phase_exp(t, U, V2, E3, None, 2, Dh_in=D1h)
                elif CFG["d2_dma"]:
                    yield from d2_dma_tree(t, U, V2, E3, E2)
                else:
                    yield from d_phase_exp(t, U, V2, E3, E2, 2)
                main_tile.state[t] = (U, E3)
            main_tile.state = {}

            def tail_tile(t, gated=False):
                if gated:
                    # spin (cost-model no-ops) until our main has finished
                    # emitting; lets the last tails share their mains' drain
                    while t not in main_tile.state:
                        yield {"nop": 50}
                U, E3 = main_tile.state.pop(t)
                yield from softmax_r(t, E3, 3)
                R3 = softmax_r.out
                yield from s_phase(t, U, R3, 3)
                SB = s_phase.out
                SBt3 = s_phase.out_tile
                Sv = SB.rearrange("p (n c) -> p n c", n=NO)
                yield from squash(t, Sv, 3, f32, CFG["newton3"])
                emit_out(t, squash.out, SBt3)
                yield _merge(_pe(256, 2), _act(256))

            def drain(gens):
                # Greedy list scheduler over generator segments: advance the
                # generator whose next (already-known) segment completes
                # earliest on a virtual per-engine timeline.  A segment that
                # consumes another engine's result pays a semaphore-handoff
                # latency before it can start.
                HO = CFG["handoff"]
                rng = drain.rng
                jit = CFG["jit"] if CFG["seed"] else 0.0

                def jitter(tag):
                    if not jit:
                        return tag
                    return {e: ns * (1 + rng.uniform(-jit, jit))
                            for e, ns in tag.items()}

                vclock = dict(drain.vclock)
                state = []
                for g in gens:
                    try:
                        tag = jitter(next(g))
                        state.append([g, tag, 0.0, frozenset(tag)])
                    except StopIteration:
                        pass
                while state:
                    best, best_end = None, None
                    for st in state:
                        g, tag, t0, prev = st
                        dep = t0 + (HO if not set(tag) <= prev else 0)
                        start = max([dep] + [vclock.get(e, 0.0)
                                             for e in tag])
                        end = start + max(tag.values()) if tag else start
                        if best_end is None or end < best_end:
                            best, best_end = st, end
                    g, tag, t0, prev = best
                    dep = t0 + (HO if not set(tag) <= prev else 0)
                    start = max([dep] + [vclock.get(e, 0.0) for e in tag])
                    for e, ns in tag.items():
                        vclock[e] = max(vclock.get(e, 0.0), start) + ns
                    best[2] = start + (max(tag.values()) if tag else 0.0)
                    best[3] = frozenset(tag)
                    try:
                        best[1] = jitter(next(g))
                    except StopIteration:
                        state.remove(best)
                drain.vclock = vclock
                global _LAST_VCLOCK
                _LAST_VCLOCK = dict(vclock)
            drain.vclock = {}
            import random as _random
            drain.rng = _random.Random(CFG["seed"] or 1)

            # Software pipeline: tail(g-1) || main(g) || conv(g+1)
            groups = CFG.get("groups", [(0, 1), (2, 3), (4, 5), (6, 7)])
            last = len(groups) - 1
            if CFG["fused_head"]:
                # head fill: mains(0,1) gated into the first drain so the
                # routing of tile 0 overlaps the evacuation of tile 1
                drain([conv_tile(t) for t in groups[0]]
                      + [main_tile(t, gated=True) for t in groups[0]]
                      + [conv_tile(t) for t in groups[1]])
                start_gi = 1
            else:
                drain([conv_tile(t) for t in groups[0]])
                start_gi = 0
            for gi, grp in enumerate(groups):
                if gi < start_gi:
                    continue
                gens = [main_tile(t) for t in grp]
                if gi > 0:
                    gens += [tail_tile(t) for t in groups[gi - 1]]
                if gi + 1 < len(groups):
                    gens += [conv_tile(t) for t in groups[gi + 1]]
                if gi == last and CFG["gated_tail"]:
                    gens += [tail_tile(t, gated=True) for t in grp]
                drain(gens)
            if not CFG["gated_tail"]:
                drain([tail_tile(t) for t in groups[-1]])

    nc.compile()
    _BUILT[key] = nc
    return nc


def _assemble(out_halves_all):
    o = out_halves_all.reshape(-1, 2, 4, CO, NPIX)
    return np.ascontiguousarray(
        o.transpose(0, 3, 1, 2, 4).reshape(-1, CO, NO, H, W)
    )


def kernel(x, conv_w, bias):
    import sys
    if "/opt/trn_rl_repo" not in sys.path:
        sys.path.insert(0, "/opt/trn_rl_repo")
    from concourse import bass_utils

    patches, w_m, bias_bc, ident, identb = _host_prep(x, conv_w, bias)
    nc = _build_nc()
    in_maps = [
        {"patches": patches[b], "w": w_m, "bias": bias_bc, "ident": ident,
         "identb": identb}
        for b in range(BS)
    ]
    res = bass_utils.run_bass_kernel_spmd(nc, in_maps, core_ids=list(range(BS)))
    outs = np.stack([r["out"] for r in res.results])
    return _assemble(outs).astype(np.float32)
